# revision 27
# baseline (speedup 1.0000x reference)
"""DNN beamformer (MVDR with attention reference) on 8 Trainium2 NeuronCores.

Sharding: batch-parallel — core b handles batch b (B=8). Per core:
  - attention: psd_s -> feat -> MLP -> softmax -> u (1,6)
  - 257 independent 6x6 complex solves psd_n X = psd_s via Gauss-Jordan,
    vectorized with (b,f) pairs on SBUF partitions (3 groups of 120 f)
  - trace-normalize, steer by u -> beamforming weights
  - apply y = w^H x via block-diagonal PE matmuls (10 freqs per matmul)
"""
import sys
import numpy as np

for _p in ("/opt/trn_rl_repo",):
    if _p not in sys.path:
        sys.path.insert(0, _p)

import concourse.bacc as bacc
import concourse.mybir as mybir
import concourse.tile as tile
from concourse.bass_types import AP
from concourse import bass_utils

F32 = mybir.dt.float32
F32R = mybir.dt.float32r
Alu = mybir.AluOpType
Act = mybir.ActivationFunctionType
AX = mybir.AxisListType

B, F, C, T, ATT = 8, 257, 6, 1500, 320
FP = 120            # freqs per partition-group
G = 3               # partition groups (3*120 = 360 >= 257)
NU = 10             # freqs per PE block
NB = 26             # number of PE blocks (25*10 + 7 = 257)
NSC = 12            # 2 (re/im) * C rows per freq in the block-diag
TCH = [(0, 512), (512, 512), (1024, 476)]      # psum-bank-aligned T chunks
BCH = [(0, 5), (5, 10), (10, 14), (14, 18), (18, 22), (22, 26)]  # rhs block chunks
# 4 blocks per PSUM tile (PE out base partition must be 0/32/64/96)
SUP = [list(range(a, min(a + 4, NB))) for a in range(0, NB, 4)]

_CACHED = None


def _build():
    nc = bacc.Bacc("TRN2", target_bir_lowering=False, debug=False,
                   enable_asserts=False, num_devices=8)

    data_r = nc.dram_tensor("data_r", [F, C, T], F32R, kind="ExternalInput")
    data_i = nc.dram_tensor("data_i", [F, C, T], F32R, kind="ExternalInput")
    aug_r = nc.dram_tensor("aug_r", [128, G, 6, 12], F32, kind="ExternalInput")
    aug_i = nc.dram_tensor("aug_i", [128, G, 6, 12], F32, kind="ExternalInput")
    wmlp_in = nc.dram_tensor("wmlp_in", [128, G, ATT], F32, kind="ExternalInput")
    wg_rep = nc.dram_tensor("wg_rep", [6, ATT], F32, kind="ExternalInput")
    b_rep = nc.dram_tensor("b_rep", [6, ATT], F32, kind="ExternalInput")
    ones128 = nc.dram_tensor("ones128", [1, 128], F32, kind="ExternalInput")
    ident6 = nc.dram_tensor("ident6", [6, 6], F32, kind="ExternalInput")
    ident26 = nc.dram_tensor("ident26", [26, 26], F32, kind="ExternalInput")
    mask10_in = nc.dram_tensor("mask10_in", [120, NU], mybir.dt.uint8, kind="ExternalInput")

    enh_r = nc.dram_tensor("enh_r", [F, T], F32, kind="ExternalOutput")
    enh_i = nc.dram_tensor("enh_i", [F, T], F32, kind="ExternalOutput")
    u_out = nc.dram_tensor("u_out", [1, 6], F32, kind="ExternalOutput")

    with tile.TileContext(nc) as tc:
        _emit(tc, data_r, data_i, aug_r, aug_i, wmlp_in, wg_rep, b_rep,
              ones128, ident6, ident26, mask10_in, enh_r, enh_i, u_out)

    nc.compile()
    return nc


def _emit(tc, data_r, data_i, aug_r, aug_i, wmlp_in, wg_rep, b_rep,
          ones128, ident6, ident26, mask10_in, enh_r, enh_i, u_out):
    nc = tc.nc
    from contextlib import ExitStack
    ctx = ExitStack()

    cst = ctx.enter_context(tc.tile_pool(name="cst", bufs=1))
    small = ctx.enter_context(tc.tile_pool(name="small", bufs=1))
    tmp4 = ctx.enter_context(tc.tile_pool(name="tmp4", bufs=8))
    rhsp = ctx.enter_context(tc.tile_pool(name="rhsp", bufs=3))
    stgp = ctx.enter_context(tc.tile_pool(name="stgp", bufs=2))
    dramp = ctx.enter_context(tc.tile_pool(name="dramp", bufs=1, space="DRAM"))
    psum_s = ctx.enter_context(tc.tile_pool(name="psum_s", bufs=2, space="PSUM"))
    psum_m = ctx.enter_context(tc.tile_pool(name="psum_m", bufs=2, space="PSUM"))

    # ---------------- constant-ish loads ----------------
    augR = cst.tile([128, G, 6, 12], F32)
    augI = cst.tile([128, G, 6, 12], F32)
    wmlp = cst.tile([128, G, ATT], F32)
    wgr = cst.tile([6, ATT], F32)
    brp = cst.tile([6, ATT], F32)
    ones = cst.tile([1, 128], F32)
    id6 = cst.tile([6, 6], F32)
    id26 = cst.tile([26, 26], F32)
    mask10 = cst.tile([120, NU], mybir.dt.uint8)
    nc.sync.dma_start(augR[:], aug_r[:])
    nc.sync.dma_start(augI[:], aug_i[:])
    nc.sync.dma_start(wmlp[:], wmlp_in[:])
    nc.sync.dma_start(wgr[:], wg_rep[:])
    nc.sync.dma_start(brp[:], b_rep[:])
    nc.sync.dma_start(ones[:], ones128[:])
    nc.sync.dma_start(id6[:], ident6[:])
    nc.sync.dma_start(id26[:], ident26[:])
    nc.sync.dma_start(mask10[:], mask10_in[:])

    def diag_ap(t, col0):
        # [128, G, 6] strided diagonal of the 6x12 augmented row-major block
        base = t[:]
        return AP(tensor=base.tensor, offset=base.offset + col0,
                  ap=[list(base.ap[0]), list(base.ap[1]), [13, 6]])

    # ---------------- rhs data loads (start streaming immediately) ---------
    # rhs row k = 60s + 6u + c holds data[10b+u, c] (s=0 real, s=1 imag).
    # Within an s-half the partition index is affine in the source address
    # (addr = 1500*k), so one DMA covers 60 partitions -> all SDMA engines.
    rhs_tiles = {}
    for ci, (b0, b1) in enumerate(BCH):
        nbc = b1 - b0
        pitch = nbc * T
        rt = rhsp.tile([120, nbc * T], F32R, tag="rhs")
        rhs_tiles[ci] = rt
        rta = rt[:]
        nfull = nbc - 1 if b1 == NB else nbc  # block 25 has only 7 valid freqs
        for s, dram in ((0, data_r), (1, data_i)):
            eng = (nc.sync, nc.scalar, nc.gpsimd)[(2 * ci + s) % 3]
            dst = AP(tensor=rta.tensor, offset=rta.offset + (60 * s) * pitch,
                     ap=[[pitch, 60], [T, nfull], [1, T]])
            src = AP(tensor=dram[:].tensor, offset=(10 * b0) * C * T,
                     ap=[[T, 60], [10 * C * T, nfull], [1, T]])
            eng.dma_start(dst, src)
            if nfull < nbc:
                # block 25: 7 valid freqs (42 rows) + 18 dup rows from valid
                # data; the dup rows are annihilated by the zero lhsT columns.
                dst = AP(tensor=rta.tensor,
                         offset=rta.offset + (60 * s) * pitch + nfull * T,
                         ap=[[pitch, 42], [1, T]])
                src = AP(tensor=dram[:].tensor, offset=250 * C * T,
                         ap=[[T, 42], [1, T]])
                nc.sync.dma_start(dst, src)
                dst = AP(tensor=rta.tensor,
                         offset=rta.offset + (60 * s + 42) * pitch + nfull * T,
                         ap=[[pitch, 18], [1, T]])
                src = AP(tensor=dram[:].tensor, offset=247 * C * T,
                         ap=[[T, 18], [1, T]])
                nc.sync.dma_start(dst, src)

    # ---------------- attention: feat -> MLP -> softmax -> u ----------------
    rsR = small.tile([128, G, 6], F32)
    rsI = small.tile([128, G, 6], F32)
    sq = small.tile([128, G, 6], F32)
    sq2 = small.tile([128, G, 6], F32)
    feat = small.tile([128, G, 6], F32)
    nc.vector.tensor_reduce(rsR[:], augR[:, :, :, 6:12], AX.X, Alu.add)
    nc.vector.tensor_tensor(rsR[:], rsR[:], diag_ap(augR, 6), Alu.subtract)
    nc.vector.tensor_reduce(rsI[:], augI[:, :, :, 6:12], AX.X, Alu.add)
    nc.vector.tensor_tensor(rsI[:], rsI[:], diag_ap(augI, 6), Alu.subtract)
    nc.vector.tensor_tensor(sq[:], rsR[:], rsR[:], Alu.mult)
    nc.vector.tensor_tensor(sq2[:], rsI[:], rsI[:], Alu.mult)
    nc.vector.tensor_tensor(sq[:], sq[:], sq2[:], Alu.add)
    nc.scalar.activation(feat[:], sq[:], Act.Sqrt, bias=0.0, scale=1.0 / 25.0)

    mlp_ps = psum_s.tile([6, ATT], F32, tag="sp")
    for g in range(G):
        nc.tensor.matmul(mlp_ps[:], feat[:, g, :], wmlp[:, g, :],
                         start=(g == 0), stop=(g == G - 1))
    tb = small.tile([6, ATT], F32)
    nc.vector.tensor_tensor(tb[:], mlp_ps[:], brp[:], Alu.add)
    th = small.tile([6, ATT], F32)
    nc.scalar.activation(th[:], tb[:], Act.Tanh)
    tm = small.tile([6, ATT], F32)
    nc.vector.tensor_tensor(tm[:], th[:], wgr[:], Alu.mult)
    e_sb = small.tile([6, 1], F32)
    nc.vector.tensor_reduce(e_sb[:], tm[:], AX.X, Alu.add)

    eT_ps = psum_s.tile([1, 6], F32, tag="sp")
    nc.tensor.matmul(eT_ps[:], e_sb[:], id6[:], is_transpose=True,
                     start=True, stop=True)
    mx = small.tile([1, 1], F32)
    mb = small.tile([1, 1], F32)
    ex = small.tile([1, 6], F32)
    sm = small.tile([1, 1], F32)
    rc = small.tile([1, 1], F32)
    u_sb = small.tile([1, 6], F32)
    nc.vector.tensor_reduce(mx[:], eT_ps[:], AX.X, Alu.max)
    nc.vector.tensor_scalar_mul(mb[:], mx[:], -2.0)
    nc.scalar.activation(ex[:], eT_ps[:], Act.Exp, bias=mb[:], scale=2.0)
    nc.vector.tensor_reduce(sm[:], ex[:], AX.X, Alu.add)
    nc.vector.reciprocal(rc[:], sm[:])
    nc.vector.tensor_scalar_mul(u_sb[:], ex[:], rc[:])
    nc.sync.dma_start(u_out[:], u_sb[:])

    urep_ps = psum_s.tile([128, 6], F32, tag="sp")
    nc.tensor.matmul(urep_ps[:], ones[:], u_sb[:], start=True, stop=True)
    u_rep = small.tile([128, 6], F32)
    nc.vector.tensor_copy(u_rep[:], urep_ps[:])

    # ---------------- Gauss-Jordan elimination (augmented [A | B]) ---------
    invd = small.tile([128, G, 6], F32)
    fr = small.tile([128, G, 6], F32)
    fi = small.tile([128, G, 6], F32)
    for i in range(6):
        w = 11 - i
        nc.vector.reciprocal(invd[:, :, i], augR[:, :, i, i])
        ird_b = invd[:, :, i].unsqueeze(2).broadcast_to([128, G, 6])
        nc.vector.tensor_tensor(fr[:], augR[:, :, :, i], ird_b, Alu.mult)
        nc.vector.tensor_tensor(fi[:], augI[:, :, :, i], ird_b, Alu.mult)
        nc.vector.memset(fr[:, :, i], 0.0)
        nc.vector.memset(fi[:, :, i], 0.0)
        rowR = augR[:, :, i, i + 1:].unsqueeze(2).broadcast_to([128, G, 6, w])
        rowI = augI[:, :, i, i + 1:].unsqueeze(2).broadcast_to([128, G, 6, w])
        fr_b = fr[:].unsqueeze(3).broadcast_to([128, G, 6, w])
        fi_b = fi[:].unsqueeze(3).broadcast_to([128, G, 6, w])
        t1 = tmp4.tile([128, G, 6, w], F32, tag="t1")
        t2 = tmp4.tile([128, G, 6, w], F32, tag="t2")
        t3 = tmp4.tile([128, G, 6, w], F32, tag="t3")
        t4 = tmp4.tile([128, G, 6, w], F32, tag="t4")
        nc.vector.tensor_tensor(t1[:], fr_b, rowR, Alu.mult)
        nc.vector.tensor_tensor(t2[:], fi_b, rowI, Alu.mult)
        nc.vector.tensor_tensor(t3[:], fr_b, rowI, Alu.mult)
        nc.vector.tensor_tensor(t4[:], fi_b, rowR, Alu.mult)
        nc.vector.tensor_tensor(augR[:, :, :, i + 1:], augR[:, :, :, i + 1:], t1[:], Alu.subtract)
        nc.vector.tensor_tensor(augR[:, :, :, i + 1:], augR[:, :, :, i + 1:], t2[:], Alu.add)
        nc.vector.tensor_tensor(augI[:, :, :, i + 1:], augI[:, :, :, i + 1:], t3[:], Alu.subtract)
        nc.vector.tensor_tensor(augI[:, :, :, i + 1:], augI[:, :, :, i + 1:], t4[:], Alu.subtract)

    # ---------------- trace, steering, weights ----------------
    t6a = small.tile([128, G, 6], F32)
    t6b = small.tile([128, G, 6], F32)
    trR = small.tile([128, G], F32)
    trI = small.tile([128, G], F32)
    nc.vector.tensor_tensor(t6a[:], diag_ap(augR, 6), invd[:], Alu.mult)
    nc.vector.tensor_reduce(trR[:], t6a[:], AX.X, Alu.add)
    nc.vector.tensor_tensor(t6b[:], diag_ap(augI, 6), invd[:], Alu.mult)
    nc.vector.tensor_reduce(trI[:], t6b[:], AX.X, Alu.add)

    u_b = u_rep[:].unsqueeze(1).unsqueeze(2).broadcast_to([128, G, 6, 6])
    t66 = tmp4.tile([128, G, 6, 6], F32, tag="t1")
    rawR = small.tile([128, G, 6], F32)
    rawI = small.tile([128, G, 6], F32)
    nc.vector.tensor_tensor(t66[:], augR[:, :, :, 6:12], u_b, Alu.mult)
    nc.vector.tensor_reduce(rawR[:], t66[:], AX.X, Alu.add)
    t66b = tmp4.tile([128, G, 6, 6], F32, tag="t2")
    nc.vector.tensor_tensor(t66b[:], augI[:, :, :, 6:12], u_b, Alu.mult)
    nc.vector.tensor_reduce(rawI[:], t66b[:], AX.X, Alu.add)
    nc.vector.tensor_tensor(rawR[:], rawR[:], invd[:], Alu.mult)
    nc.vector.tensor_tensor(rawI[:], rawI[:], invd[:], Alu.mult)

    trr = small.tile([128, G], F32)
    den = small.tile([128, G], F32)
    dn2 = small.tile([128, G], F32)
    rec = small.tile([128, G], F32)
    itr = small.tile([128, G], F32)
    iti = small.tile([128, G], F32)
    nc.vector.tensor_scalar_add(trr[:], trR[:], 1e-6)
    nc.vector.tensor_tensor(den[:], trr[:], trr[:], Alu.mult)
    nc.vector.tensor_tensor(dn2[:], trI[:], trI[:], Alu.mult)
    nc.vector.tensor_tensor(den[:], den[:], dn2[:], Alu.add)
    nc.vector.reciprocal(rec[:], den[:])
    nc.vector.tensor_tensor(itr[:], trr[:], rec[:], Alu.mult)
    nc.vector.scalar_tensor_tensor(iti[:], trI[:], -1.0, rec[:], Alu.mult, Alu.mult)

    itr_b = itr[:].unsqueeze(2).broadcast_to([128, G, 6])
    iti_b = iti[:].unsqueeze(2).broadcast_to([128, G, 6])
    # wcat[p, g, rho, sc]: rho=0 -> [a | b], rho=1 -> [-b | a]
    wcat = small.tile([128, G, 2, NSC], F32)
    m3 = small.tile([128, G, 6], F32)
    m4 = small.tile([128, G, 6], F32)
    nc.vector.tensor_tensor(m3[:], rawR[:], itr_b, Alu.mult)
    nc.vector.tensor_tensor(m4[:], rawI[:], iti_b, Alu.mult)
    nc.vector.tensor_tensor(wcat[:, :, 0, 0:6], m3[:], m4[:], Alu.subtract)   # a
    nc.vector.tensor_tensor(m3[:], rawR[:], iti_b, Alu.mult)
    nc.vector.tensor_tensor(m4[:], rawI[:], itr_b, Alu.mult)
    nc.vector.tensor_tensor(wcat[:, :, 0, 6:12], m3[:], m4[:], Alu.add)       # b
    nc.vector.tensor_scalar_mul(wcat[:, :, 1, 0:6], wcat[:, :, 0, 6:12], -1.0)  # -b
    nc.vector.tensor_copy(wcat[:, :, 1, 6:12], wcat[:, :, 0, 0:6])            # a

    # ------- block-diag lhsT: DRAM round-trip + PE transpose + masked copy --
    # bd3[f, rho, sc] = wcat[f%120, f//120, rho, sc]
    bd3 = dramp.tile([G * FP, 2, NSC], F32)
    bda = bd3[:]
    bdst = AP(tensor=bda.tensor, offset=bda.offset,
              ap=[[24, 120], [FP * 24, G], [1, 24]])
    nc.sync.dma_start(bdst, wcat[0:120, :, :, :])

    # inb_rho[b, 12u+sc] = w[10b+u, rho, sc];  wT_rho[k, b] = inb_rho[b, k]
    lhsTf = cst.tile([120, NB * 32], F32)
    lhsT = cst.tile([120, NB * 32], F32R)
    nc.vector.memset(lhsTf[:], 0.0)
    lta = lhsTf[:]
    mask_b = mask10[:].unsqueeze(1).broadcast_to([120, NB, NU])
    # one contiguous load of bd3 (960B descriptors), then a strided DVE
    # shuffle into the transpose-input order k = 60s + 6u + c
    binb = small.tile([NB, NU * 24], F32)
    bsrc = AP(tensor=bda.tensor, offset=bda.offset,
              ap=[[NU * 24, NB], [1, NU * 24]])
    nc.sync.dma_start(binb[:], bsrc)
    for rho in range(2):
        inb = small.tile([NB, FP], F32, tag=f"inb{rho}")
        iap = AP(tensor=inb[:].tensor, offset=inb[:].offset,
                 ap=[list(inb[:].ap[0]), [60, 2], [6, NU], [1, 6]])
        bap = AP(tensor=binb[:].tensor, offset=binb[:].offset + NSC * rho,
                 ap=[list(binb[:].ap[0]), [6, 2], [24, NU], [1, 6]])
        nc.vector.tensor_copy(iap, bap)
        psT = psum_s.tile([120, NB], F32, tag="sp")
        nc.tensor.matmul(psT[:], inb[:], id26[:], is_transpose=True,
                         start=True, stop=True)
        wT = small.tile([120, NB], F32, tag=f"wT{rho}")
        nc.vector.tensor_copy(wT[:], psT[:])
        out_ap = AP(tensor=lta.tensor, offset=lta.offset + 16 * rho,
                    ap=[[NB * 32, 120], [32, NB], [1, NU]])
        data_ap = wT[:].unsqueeze(2).broadcast_to([120, NB, NU])
        nc.vector.copy_predicated(out_ap, mask_b, data_ap)
    nc.vector.tensor_copy(lhsT[:], lhsTf[:])

    # ---------------- beamforming application (PE block-diag matmuls) ------
    # fp32r matmuls require dst start_partition == 0 -> one PSUM tile per
    # block; outputs of 6 blocks are staged side by side so each out-DMA
    # spans many descriptors (keeps the SDMA engines fed).
    for s0 in range(0, NB, 6):
        sblocks = list(range(s0, min(s0 + 6, NB)))
        stg = stgp.tile([32, 6 * 1500], F32, tag="stg")
        for b in sblocks:
            j = b - s0
            ci = next(k for k, (a0, a1) in enumerate(BCH) if a0 <= b < a1)
            bl = b - BCH[ci][0]
            rt = rhs_tiles[ci]
            ps = psum_m.tile([32, 1536], F32, tag="mac")
            for (t0, tn) in TCH:
                lhs_s = lhsT[:, 32 * b:32 * (b + 1)]
                rhs_s = rt[:, bl * T + t0: bl * T + t0 + tn]
                nc.tensor.matmul(ps[:, t0:t0 + tn], lhs_s, rhs_s,
                                 start=True, stop=True)
            # split the PSUM->SBUF copy across ACT and DVE so neither
            # engine paces the PE
            nc.scalar.activation(stg[0:26, j * 1500: j * 1500 + 768],
                                 ps[0:26, 0:768], Act.Copy)
            nc.vector.tensor_copy(stg[0:26, j * 1500 + 768: j * 1500 + T],
                                  ps[0:26, 768:T])
        nfb = len([b for b in sblocks if b != NB - 1])  # full blocks
        sta = stg[:]
        for rho, dram in ((0, enh_r), (1, enh_i)):
            eng = (nc.sync, nc.scalar)[(s0 // 6 + rho) % 2]
            src = AP(tensor=sta.tensor, offset=sta.offset + (16 * rho) * (6 * 1500),
                     ap=[[6 * 1500, NU], [1500, nfb], [1, T]])
            dst = AP(tensor=dram[:].tensor, offset=(10 * s0) * T,
                     ap=[[T, NU], [10 * T, nfb], [1, T]])
            eng.dma_start(dst, src)
        if NB - 1 in sblocks:
            j = NB - 1 - s0
            for rho, dram in ((0, enh_r), (1, enh_i)):
                src = AP(tensor=sta.tensor,
                         offset=sta.offset + (16 * rho) * (6 * 1500) + j * 1500,
                         ap=[[6 * 1500, 7], [1, T]])
                dst = AP(tensor=dram[:].tensor, offset=(10 * (NB - 1)) * T,
                         ap=[[T, 7], [1, T]])
                nc.sync.dma_start(dst, src)

    ctx.close()


def _host_prep(inputs, core):
    """Build the per-core input map from the full problem inputs."""
    psd_s_r = np.asarray(inputs["psd_s_r"][core])
    psd_s_i = np.asarray(inputs["psd_s_i"][core])
    psd_n_r = np.asarray(inputs["psd_n_r"][core])
    psd_n_i = np.asarray(inputs["psd_n_i"][core])
    w_mlp = np.asarray(inputs["w_mlp"])
    b_mlp = np.asarray(inputs["b_mlp"])
    w_gvec = np.asarray(inputs["w_gvec"])

    aug_r = np.zeros((G, 128, 6, 12), np.float32)
    aug_i = np.zeros((G, 128, 6, 12), np.float32)
    aug_r[:, :, :, 0:6] = np.eye(6, dtype=np.float32)  # pad A with identity
    fpad_r = np.zeros((G * FP, 6, 6), np.float32)
    fpad_r[:F] = psd_n_r
    fpad_i = np.zeros((G * FP, 6, 6), np.float32)
    fpad_i[:F] = psd_n_i
    fpad_r[F:] = np.eye(6, dtype=np.float32)
    aug_r[:, :FP, :, 0:6] = fpad_r.reshape(G, FP, 6, 6)
    aug_i[:, :FP, :, 0:6] = fpad_i.reshape(G, FP, 6, 6)
    spad_r = np.zeros((G * FP, 6, 6), np.float32)
    spad_r[:F] = psd_s_r
    spad_i = np.zeros((G * FP, 6, 6), np.float32)
    spad_i[:F] = psd_s_i
    aug_r[:, :FP, :, 6:12] = spad_r.reshape(G, FP, 6, 6)
    aug_i[:, :FP, :, 6:12] = spad_i.reshape(G, FP, 6, 6)
    aug_r = np.ascontiguousarray(aug_r.transpose(1, 0, 2, 3))
    aug_i = np.ascontiguousarray(aug_i.transpose(1, 0, 2, 3))

    wmlp = np.zeros((G, 128, ATT), np.float32)
    wpad = np.zeros((G * FP, ATT), np.float32)
    wpad[:F] = w_mlp
    wmlp[:, :FP] = wpad.reshape(G, FP, ATT)
    wmlp = np.ascontiguousarray(wmlp.transpose(1, 0, 2))

    return {
        "data_r": np.ascontiguousarray(inputs["data_r"][core]),
        "data_i": np.ascontiguousarray(inputs["data_i"][core]),
        "aug_r": aug_r,
        "aug_i": aug_i,
        "wmlp_in": wmlp,
        "wg_rep": np.ascontiguousarray(
            np.broadcast_to(w_gvec[:, 0][None, :], (6, ATT))).astype(np.float32),
        "b_rep": np.ascontiguousarray(
            np.broadcast_to(b_mlp[None, :], (6, ATT))).astype(np.float32),
        "ones128": np.ones((1, 128), np.float32),
        "ident6": np.eye(6, dtype=np.float32),
        "ident26": np.eye(26, dtype=np.float32),
        "mask10_in": (np.arange(NU)[None, :] ==
                      ((np.arange(120) % 60) // 6)[:, None]).astype(np.uint8),
    }


def get_nc():
    global _CACHED
    if _CACHED is None:
        _CACHED = _build()
    return _CACHED


def kernel(**inputs):
    nc = get_nc()
    in_maps = [_host_prep(inputs, core) for core in range(B)]
    res = bass_utils.run_bass_kernel_spmd(nc, in_maps, core_ids=list(range(B)))
    outs = res.results
    enh_r = np.stack([outs[b]["enh_r"] for b in range(B)])
    enh_i = np.stack([outs[b]["enh_i"] for b in range(B)])
    u = np.stack([outs[b]["u_out"][0] for b in range(B)])
    return enh_r, enh_i, u


# revision 28
# speedup vs baseline: 1.0186x; 1.0186x over previous
"""DNN beamformer (MVDR with attention reference) on 8 Trainium2 NeuronCores.

Sharding: batch-parallel — core b handles batch b (B=8). Per core:
  - attention: psd_s -> feat -> MLP -> softmax -> u (1,6)
  - 257 independent 6x6 complex solves psd_n X = psd_s via Gauss-Jordan,
    vectorized with (b,f) pairs on SBUF partitions (3 groups of 120 f)
  - trace-normalize, steer by u -> beamforming weights
  - apply y = w^H x via block-diagonal PE matmuls (10 freqs per matmul)
"""
import sys
import numpy as np

for _p in ("/opt/trn_rl_repo",):
    if _p not in sys.path:
        sys.path.insert(0, _p)

import concourse.bacc as bacc
import concourse.mybir as mybir
import concourse.tile as tile
from concourse.bass_types import AP
from concourse import bass_utils

F32 = mybir.dt.float32
F32R = mybir.dt.float32r
Alu = mybir.AluOpType
Act = mybir.ActivationFunctionType
AX = mybir.AxisListType

B, F, C, T, ATT = 8, 257, 6, 1500, 320
FP = 120            # freqs per partition-group
G = 3               # partition groups (3*120 = 360 >= 257)
NU = 10             # freqs per PE block
NB = 26             # number of PE blocks (25*10 + 7 = 257)
NSC = 12            # 2 (re/im) * C rows per freq in the block-diag
TCH = [(0, 512), (512, 512), (1024, 476)]      # psum-bank-aligned T chunks
BCH = [(0, 5), (5, 10), (10, 14), (14, 18), (18, 22), (22, 26)]  # rhs block chunks
# 4 blocks per PSUM tile (PE out base partition must be 0/32/64/96)
SUP = [list(range(a, min(a + 4, NB))) for a in range(0, NB, 4)]

_CACHED = None


def _build():
    nc = bacc.Bacc("TRN2", target_bir_lowering=False, debug=False,
                   enable_asserts=False, num_devices=8)

    data_r = nc.dram_tensor("data_r", [F, C, T], F32R, kind="ExternalInput")
    data_i = nc.dram_tensor("data_i", [F, C, T], F32R, kind="ExternalInput")
    aug_r = nc.dram_tensor("aug_r", [128, G, 6, 12], F32, kind="ExternalInput")
    aug_i = nc.dram_tensor("aug_i", [128, G, 6, 12], F32, kind="ExternalInput")
    wmlp_in = nc.dram_tensor("wmlp_in", [128, G, ATT], F32, kind="ExternalInput")
    wg_rep = nc.dram_tensor("wg_rep", [6, ATT], F32, kind="ExternalInput")
    b_rep = nc.dram_tensor("b_rep", [6, ATT], F32, kind="ExternalInput")
    ones128 = nc.dram_tensor("ones128", [1, 128], F32, kind="ExternalInput")
    ident6 = nc.dram_tensor("ident6", [6, 6], F32, kind="ExternalInput")
    ident26 = nc.dram_tensor("ident26", [26, 26], F32, kind="ExternalInput")
    mask10_in = nc.dram_tensor("mask10_in", [120, NU], mybir.dt.uint8, kind="ExternalInput")

    enh_r = nc.dram_tensor("enh_r", [F, T], F32, kind="ExternalOutput")
    enh_i = nc.dram_tensor("enh_i", [F, T], F32, kind="ExternalOutput")
    u_out = nc.dram_tensor("u_out", [1, 6], F32, kind="ExternalOutput")

    with tile.TileContext(nc) as tc:
        _emit(tc, data_r, data_i, aug_r, aug_i, wmlp_in, wg_rep, b_rep,
              ones128, ident6, ident26, mask10_in, enh_r, enh_i, u_out)

    nc.compile()
    return nc


def _emit(tc, data_r, data_i, aug_r, aug_i, wmlp_in, wg_rep, b_rep,
          ones128, ident6, ident26, mask10_in, enh_r, enh_i, u_out):
    nc = tc.nc
    from contextlib import ExitStack
    ctx = ExitStack()

    cst = ctx.enter_context(tc.tile_pool(name="cst", bufs=1))
    small = ctx.enter_context(tc.tile_pool(name="small", bufs=1))
    tmp4 = ctx.enter_context(tc.tile_pool(name="tmp4", bufs=8))
    rhsp = ctx.enter_context(tc.tile_pool(name="rhsp", bufs=3))
    stgp = ctx.enter_context(tc.tile_pool(name="stgp", bufs=2))
    dramp = ctx.enter_context(tc.tile_pool(name="dramp", bufs=1, space="DRAM"))
    psum_s = ctx.enter_context(tc.tile_pool(name="psum_s", bufs=2, space="PSUM"))
    psum_m = ctx.enter_context(tc.tile_pool(name="psum_m", bufs=2, space="PSUM"))

    # ---------------- constant-ish loads ----------------
    augR = cst.tile([128, G, 6, 12], F32)
    augI = cst.tile([128, G, 6, 12], F32)
    wmlp = cst.tile([128, G, ATT], F32)
    wgr = cst.tile([6, ATT], F32)
    brp = cst.tile([6, ATT], F32)
    ones = cst.tile([1, 128], F32)
    id6 = cst.tile([6, 6], F32)
    id26 = cst.tile([26, 26], F32)
    mask10 = cst.tile([120, NU], mybir.dt.uint8)
    nc.scalar.dma_start(augR[:], aug_r[:])
    nc.scalar.dma_start(augI[:], aug_i[:])
    nc.scalar.dma_start(wmlp[:], wmlp_in[:])
    nc.scalar.dma_start(wgr[:], wg_rep[:])
    nc.scalar.dma_start(brp[:], b_rep[:])
    nc.scalar.dma_start(ones[:], ones128[:])
    nc.scalar.dma_start(id6[:], ident6[:])
    nc.scalar.dma_start(id26[:], ident26[:])
    nc.scalar.dma_start(mask10[:], mask10_in[:])

    def diag_ap(t, col0):
        # [128, G, 6] strided diagonal of the 6x12 augmented row-major block
        base = t[:]
        return AP(tensor=base.tensor, offset=base.offset + col0,
                  ap=[list(base.ap[0]), list(base.ap[1]), [13, 6]])

    # ---------------- rhs data loads (start streaming immediately) ---------
    # rhs row k = 60s + 6u + c holds data[10b+u, c] (s=0 real, s=1 imag).
    # Within an s-half the partition index is affine in the source address
    # (addr = 1500*k), so one DMA covers 60 partitions -> all SDMA engines.
    rhs_tiles = {}
    for ci, (b0, b1) in enumerate(BCH):
        nbc = b1 - b0
        pitch = nbc * T
        rt = rhsp.tile([120, nbc * T], F32R, tag="rhs")
        rhs_tiles[ci] = rt
        rta = rt[:]
        nfull = nbc - 1 if b1 == NB else nbc  # block 25 has only 7 valid freqs
        for s, dram in ((0, data_r), (1, data_i)):
            eng = (nc.sync, nc.gpsimd)[(ci + s) % 2]
            dst = AP(tensor=rta.tensor, offset=rta.offset + (60 * s) * pitch,
                     ap=[[pitch, 60], [T, nfull], [1, T]])
            src = AP(tensor=dram[:].tensor, offset=(10 * b0) * C * T,
                     ap=[[T, 60], [10 * C * T, nfull], [1, T]])
            eng.dma_start(dst, src)
            if nfull < nbc:
                # block 25: 7 valid freqs (42 rows) + 18 dup rows from valid
                # data; the dup rows are annihilated by the zero lhsT columns.
                dst = AP(tensor=rta.tensor,
                         offset=rta.offset + (60 * s) * pitch + nfull * T,
                         ap=[[pitch, 42], [1, T]])
                src = AP(tensor=dram[:].tensor, offset=250 * C * T,
                         ap=[[T, 42], [1, T]])
                nc.sync.dma_start(dst, src)
                dst = AP(tensor=rta.tensor,
                         offset=rta.offset + (60 * s + 42) * pitch + nfull * T,
                         ap=[[pitch, 18], [1, T]])
                src = AP(tensor=dram[:].tensor, offset=247 * C * T,
                         ap=[[T, 18], [1, T]])
                nc.sync.dma_start(dst, src)

    # ---------------- attention: feat -> MLP -> softmax -> u ----------------
    rsR = small.tile([128, G, 6], F32)
    rsI = small.tile([128, G, 6], F32)
    sq = small.tile([128, G, 6], F32)
    sq2 = small.tile([128, G, 6], F32)
    feat = small.tile([128, G, 6], F32)
    nc.vector.tensor_reduce(rsR[:], augR[:, :, :, 6:12], AX.X, Alu.add)
    nc.vector.tensor_tensor(rsR[:], rsR[:], diag_ap(augR, 6), Alu.subtract)
    nc.vector.tensor_reduce(rsI[:], augI[:, :, :, 6:12], AX.X, Alu.add)
    nc.vector.tensor_tensor(rsI[:], rsI[:], diag_ap(augI, 6), Alu.subtract)
    nc.vector.tensor_tensor(sq[:], rsR[:], rsR[:], Alu.mult)
    nc.vector.tensor_tensor(sq2[:], rsI[:], rsI[:], Alu.mult)
    nc.vector.tensor_tensor(sq[:], sq[:], sq2[:], Alu.add)
    nc.scalar.activation(feat[:], sq[:], Act.Sqrt, bias=0.0, scale=1.0 / 25.0)

    mlp_ps = psum_s.tile([6, ATT], F32, tag="sp")
    for g in range(G):
        nc.tensor.matmul(mlp_ps[:], feat[:, g, :], wmlp[:, g, :],
                         start=(g == 0), stop=(g == G - 1))
    tb = small.tile([6, ATT], F32)
    nc.vector.tensor_tensor(tb[:], mlp_ps[:], brp[:], Alu.add)
    th = small.tile([6, ATT], F32)
    nc.scalar.activation(th[:], tb[:], Act.Tanh)
    tm = small.tile([6, ATT], F32)
    nc.vector.tensor_tensor(tm[:], th[:], wgr[:], Alu.mult)
    e_sb = small.tile([6, 1], F32)
    nc.vector.tensor_reduce(e_sb[:], tm[:], AX.X, Alu.add)

    eT_ps = psum_s.tile([1, 6], F32, tag="sp")
    nc.tensor.matmul(eT_ps[:], e_sb[:], id6[:], is_transpose=True,
                     start=True, stop=True)
    mx = small.tile([1, 1], F32)
    mb = small.tile([1, 1], F32)
    ex = small.tile([1, 6], F32)
    sm = small.tile([1, 1], F32)
    rc = small.tile([1, 1], F32)
    u_sb = small.tile([1, 6], F32)
    nc.vector.tensor_reduce(mx[:], eT_ps[:], AX.X, Alu.max)
    nc.vector.tensor_scalar_mul(mb[:], mx[:], -2.0)
    nc.scalar.activation(ex[:], eT_ps[:], Act.Exp, bias=mb[:], scale=2.0)
    nc.vector.tensor_reduce(sm[:], ex[:], AX.X, Alu.add)
    nc.vector.reciprocal(rc[:], sm[:])
    nc.vector.tensor_scalar_mul(u_sb[:], ex[:], rc[:])
    nc.scalar.dma_start(u_out[:], u_sb[:])

    urep_ps = psum_s.tile([128, 6], F32, tag="sp")
    nc.tensor.matmul(urep_ps[:], ones[:], u_sb[:], start=True, stop=True)
    u_rep = small.tile([128, 6], F32)
    nc.vector.tensor_copy(u_rep[:], urep_ps[:])

    # ---------------- Gauss-Jordan elimination (augmented [A | B]) ---------
    invd = small.tile([128, G, 6], F32)
    fr = small.tile([128, G, 6], F32)
    fi = small.tile([128, G, 6], F32)
    for i in range(6):
        w = 11 - i
        nc.vector.reciprocal(invd[:, :, i], augR[:, :, i, i])
        ird_b = invd[:, :, i].unsqueeze(2).broadcast_to([128, G, 6])
        nc.vector.tensor_tensor(fr[:], augR[:, :, :, i], ird_b, Alu.mult)
        nc.vector.tensor_tensor(fi[:], augI[:, :, :, i], ird_b, Alu.mult)
        nc.vector.memset(fr[:, :, i], 0.0)
        nc.vector.memset(fi[:, :, i], 0.0)
        rowR = augR[:, :, i, i + 1:].unsqueeze(2).broadcast_to([128, G, 6, w])
        rowI = augI[:, :, i, i + 1:].unsqueeze(2).broadcast_to([128, G, 6, w])
        fr_b = fr[:].unsqueeze(3).broadcast_to([128, G, 6, w])
        fi_b = fi[:].unsqueeze(3).broadcast_to([128, G, 6, w])
        t1 = tmp4.tile([128, G, 6, w], F32, tag="t1")
        t2 = tmp4.tile([128, G, 6, w], F32, tag="t2")
        t3 = tmp4.tile([128, G, 6, w], F32, tag="t3")
        t4 = tmp4.tile([128, G, 6, w], F32, tag="t4")
        nc.vector.tensor_tensor(t1[:], fr_b, rowR, Alu.mult)
        nc.vector.tensor_tensor(t2[:], fi_b, rowI, Alu.mult)
        nc.vector.tensor_tensor(t3[:], fr_b, rowI, Alu.mult)
        nc.vector.tensor_tensor(t4[:], fi_b, rowR, Alu.mult)
        nc.vector.tensor_tensor(augR[:, :, :, i + 1:], augR[:, :, :, i + 1:], t1[:], Alu.subtract)
        nc.vector.tensor_tensor(augR[:, :, :, i + 1:], augR[:, :, :, i + 1:], t2[:], Alu.add)
        nc.vector.tensor_tensor(augI[:, :, :, i + 1:], augI[:, :, :, i + 1:], t3[:], Alu.subtract)
        nc.vector.tensor_tensor(augI[:, :, :, i + 1:], augI[:, :, :, i + 1:], t4[:], Alu.subtract)

    # ---------------- trace, steering, weights ----------------
    t6a = small.tile([128, G, 6], F32)
    t6b = small.tile([128, G, 6], F32)
    trR = small.tile([128, G], F32)
    trI = small.tile([128, G], F32)
    nc.vector.tensor_tensor(t6a[:], diag_ap(augR, 6), invd[:], Alu.mult)
    nc.vector.tensor_reduce(trR[:], t6a[:], AX.X, Alu.add)
    nc.vector.tensor_tensor(t6b[:], diag_ap(augI, 6), invd[:], Alu.mult)
    nc.vector.tensor_reduce(trI[:], t6b[:], AX.X, Alu.add)

    u_b = u_rep[:].unsqueeze(1).unsqueeze(2).broadcast_to([128, G, 6, 6])
    t66 = tmp4.tile([128, G, 6, 6], F32, tag="t1")
    rawR = small.tile([128, G, 6], F32)
    rawI = small.tile([128, G, 6], F32)
    nc.vector.tensor_tensor(t66[:], augR[:, :, :, 6:12], u_b, Alu.mult)
    nc.vector.tensor_reduce(rawR[:], t66[:], AX.X, Alu.add)
    t66b = tmp4.tile([128, G, 6, 6], F32, tag="t2")
    nc.vector.tensor_tensor(t66b[:], augI[:, :, :, 6:12], u_b, Alu.mult)
    nc.vector.tensor_reduce(rawI[:], t66b[:], AX.X, Alu.add)
    nc.vector.tensor_tensor(rawR[:], rawR[:], invd[:], Alu.mult)
    nc.vector.tensor_tensor(rawI[:], rawI[:], invd[:], Alu.mult)

    trr = small.tile([128, G], F32)
    den = small.tile([128, G], F32)
    dn2 = small.tile([128, G], F32)
    rec = small.tile([128, G], F32)
    itr = small.tile([128, G], F32)
    iti = small.tile([128, G], F32)
    nc.vector.tensor_scalar_add(trr[:], trR[:], 1e-6)
    nc.vector.tensor_tensor(den[:], trr[:], trr[:], Alu.mult)
    nc.vector.tensor_tensor(dn2[:], trI[:], trI[:], Alu.mult)
    nc.vector.tensor_tensor(den[:], den[:], dn2[:], Alu.add)
    nc.vector.reciprocal(rec[:], den[:])
    nc.vector.tensor_tensor(itr[:], trr[:], rec[:], Alu.mult)
    nc.vector.scalar_tensor_tensor(iti[:], trI[:], -1.0, rec[:], Alu.mult, Alu.mult)

    itr_b = itr[:].unsqueeze(2).broadcast_to([128, G, 6])
    iti_b = iti[:].unsqueeze(2).broadcast_to([128, G, 6])
    # wcat[p, g, rho, sc]: rho=0 -> [a | b], rho=1 -> [-b | a]
    wcat = small.tile([128, G, 2, NSC], F32)
    m3 = small.tile([128, G, 6], F32)
    m4 = small.tile([128, G, 6], F32)
    nc.vector.tensor_tensor(m3[:], rawR[:], itr_b, Alu.mult)
    nc.vector.tensor_tensor(m4[:], rawI[:], iti_b, Alu.mult)
    nc.vector.tensor_tensor(wcat[:, :, 0, 0:6], m3[:], m4[:], Alu.subtract)   # a
    nc.vector.tensor_tensor(m3[:], rawR[:], iti_b, Alu.mult)
    nc.vector.tensor_tensor(m4[:], rawI[:], itr_b, Alu.mult)
    nc.vector.tensor_tensor(wcat[:, :, 0, 6:12], m3[:], m4[:], Alu.add)       # b
    nc.vector.tensor_scalar_mul(wcat[:, :, 1, 0:6], wcat[:, :, 0, 6:12], -1.0)  # -b
    nc.vector.tensor_copy(wcat[:, :, 1, 6:12], wcat[:, :, 0, 0:6])            # a

    # ------- block-diag lhsT: DRAM round-trip + PE transpose + masked copy --
    # bd3[f, rho, sc] = wcat[f%120, f//120, rho, sc]
    bd3 = dramp.tile([G * FP, 2, NSC], F32)
    bda = bd3[:]
    bdst = AP(tensor=bda.tensor, offset=bda.offset,
              ap=[[24, 120], [FP * 24, G], [1, 24]])
    nc.scalar.dma_start(bdst, wcat[0:120, :, :, :])

    # inb_rho[b, 12u+sc] = w[10b+u, rho, sc];  wT_rho[k, b] = inb_rho[b, k]
    lhsTf = cst.tile([120, NB * 32], F32)
    lhsT = cst.tile([120, NB * 32], F32R)
    nc.vector.memset(lhsTf[:], 0.0)
    lta = lhsTf[:]
    mask_b = mask10[:].unsqueeze(1).broadcast_to([120, NB, NU])
    # one contiguous load of bd3 (960B descriptors), then a strided DVE
    # shuffle into the transpose-input order k = 60s + 6u + c
    binb = small.tile([NB, NU * 24], F32)
    bsrc = AP(tensor=bda.tensor, offset=bda.offset,
              ap=[[NU * 24, NB], [1, NU * 24]])
    nc.scalar.dma_start(binb[:], bsrc)
    for rho in range(2):
        inb = small.tile([NB, FP], F32, tag=f"inb{rho}")
        iap = AP(tensor=inb[:].tensor, offset=inb[:].offset,
                 ap=[list(inb[:].ap[0]), [60, 2], [6, NU], [1, 6]])
        bap = AP(tensor=binb[:].tensor, offset=binb[:].offset + NSC * rho,
                 ap=[list(binb[:].ap[0]), [6, 2], [24, NU], [1, 6]])
        nc.vector.tensor_copy(iap, bap)
        psT = psum_s.tile([120, NB], F32, tag="sp")
        nc.tensor.matmul(psT[:], inb[:], id26[:], is_transpose=True,
                         start=True, stop=True)
        wT = small.tile([120, NB], F32, tag=f"wT{rho}")
        nc.vector.tensor_copy(wT[:], psT[:])
        out_ap = AP(tensor=lta.tensor, offset=lta.offset + 16 * rho,
                    ap=[[NB * 32, 120], [32, NB], [1, NU]])
        data_ap = wT[:].unsqueeze(2).broadcast_to([120, NB, NU])
        nc.vector.copy_predicated(out_ap, mask_b, data_ap)
    nc.vector.tensor_copy(lhsT[:], lhsTf[:])

    # ---------------- beamforming application (PE block-diag matmuls) ------
    # fp32r matmuls require dst start_partition == 0 -> one PSUM tile per
    # block; outputs of 6 blocks are staged side by side so each out-DMA
    # spans many descriptors (keeps the SDMA engines fed).
    for s0 in range(0, NB, 6):
        sblocks = list(range(s0, min(s0 + 6, NB)))
        stg = stgp.tile([32, 6 * 1500], F32, tag="stg")
        for b in sblocks:
            j = b - s0
            ci = next(k for k, (a0, a1) in enumerate(BCH) if a0 <= b < a1)
            bl = b - BCH[ci][0]
            rt = rhs_tiles[ci]
            ps = psum_m.tile([32, 1536], F32, tag="mac")
            for (t0, tn) in TCH:
                lhs_s = lhsT[:, 32 * b:32 * (b + 1)]
                rhs_s = rt[:, bl * T + t0: bl * T + t0 + tn]
                nc.tensor.matmul(ps[:, t0:t0 + tn], lhs_s, rhs_s,
                                 start=True, stop=True)
            # split the PSUM->SBUF copy across ACT and DVE so neither
            # engine paces the PE
            nc.scalar.activation(stg[0:26, j * 1500: j * 1500 + 768],
                                 ps[0:26, 0:768], Act.Copy)
            nc.vector.tensor_copy(stg[0:26, j * 1500 + 768: j * 1500 + T],
                                  ps[0:26, 768:T])
        nfb = len([b for b in sblocks if b != NB - 1])  # full blocks
        sta = stg[:]
        for rho, dram in ((0, enh_r), (1, enh_i)):
            eng = (nc.sync, nc.gpsimd)[(s0 // 6 + rho) % 2]
            src = AP(tensor=sta.tensor, offset=sta.offset + (16 * rho) * (6 * 1500),
                     ap=[[6 * 1500, NU], [1500, nfb], [1, T]])
            dst = AP(tensor=dram[:].tensor, offset=(10 * s0) * T,
                     ap=[[T, NU], [10 * T, nfb], [1, T]])
            eng.dma_start(dst, src)
        if NB - 1 in sblocks:
            j = NB - 1 - s0
            for rho, dram in ((0, enh_r), (1, enh_i)):
                src = AP(tensor=sta.tensor,
                         offset=sta.offset + (16 * rho) * (6 * 1500) + j * 1500,
                         ap=[[6 * 1500, 7], [1, T]])
                dst = AP(tensor=dram[:].tensor, offset=(10 * (NB - 1)) * T,
                         ap=[[T, 7], [1, T]])
                nc.sync.dma_start(dst, src)

    ctx.close()


def _host_prep(inputs, core):
    """Build the per-core input map from the full problem inputs."""
    psd_s_r = np.asarray(inputs["psd_s_r"][core])
    psd_s_i = np.asarray(inputs["psd_s_i"][core])
    psd_n_r = np.asarray(inputs["psd_n_r"][core])
    psd_n_i = np.asarray(inputs["psd_n_i"][core])
    w_mlp = np.asarray(inputs["w_mlp"])
    b_mlp = np.asarray(inputs["b_mlp"])
    w_gvec = np.asarray(inputs["w_gvec"])

    aug_r = np.zeros((G, 128, 6, 12), np.float32)
    aug_i = np.zeros((G, 128, 6, 12), np.float32)
    aug_r[:, :, :, 0:6] = np.eye(6, dtype=np.float32)  # pad A with identity
    fpad_r = np.zeros((G * FP, 6, 6), np.float32)
    fpad_r[:F] = psd_n_r
    fpad_i = np.zeros((G * FP, 6, 6), np.float32)
    fpad_i[:F] = psd_n_i
    fpad_r[F:] = np.eye(6, dtype=np.float32)
    aug_r[:, :FP, :, 0:6] = fpad_r.reshape(G, FP, 6, 6)
    aug_i[:, :FP, :, 0:6] = fpad_i.reshape(G, FP, 6, 6)
    spad_r = np.zeros((G * FP, 6, 6), np.float32)
    spad_r[:F] = psd_s_r
    spad_i = np.zeros((G * FP, 6, 6), np.float32)
    spad_i[:F] = psd_s_i
    aug_r[:, :FP, :, 6:12] = spad_r.reshape(G, FP, 6, 6)
    aug_i[:, :FP, :, 6:12] = spad_i.reshape(G, FP, 6, 6)
    aug_r = np.ascontiguousarray(aug_r.transpose(1, 0, 2, 3))
    aug_i = np.ascontiguousarray(aug_i.transpose(1, 0, 2, 3))

    wmlp = np.zeros((G, 128, ATT), np.float32)
    wpad = np.zeros((G * FP, ATT), np.float32)
    wpad[:F] = w_mlp
    wmlp[:, :FP] = wpad.reshape(G, FP, ATT)
    wmlp = np.ascontiguousarray(wmlp.transpose(1, 0, 2))

    return {
        "data_r": np.ascontiguousarray(inputs["data_r"][core]),
        "data_i": np.ascontiguousarray(inputs["data_i"][core]),
        "aug_r": aug_r,
        "aug_i": aug_i,
        "wmlp_in": wmlp,
        "wg_rep": np.ascontiguousarray(
            np.broadcast_to(w_gvec[:, 0][None, :], (6, ATT))).astype(np.float32),
        "b_rep": np.ascontiguousarray(
            np.broadcast_to(b_mlp[None, :], (6, ATT))).astype(np.float32),
        "ones128": np.ones((1, 128), np.float32),
        "ident6": np.eye(6, dtype=np.float32),
        "ident26": np.eye(26, dtype=np.float32),
        "mask10_in": (np.arange(NU)[None, :] ==
                      ((np.arange(120) % 60) // 6)[:, None]).astype(np.uint8),
    }


def get_nc():
    global _CACHED
    if _CACHED is None:
        _CACHED = _build()
    return _CACHED


def kernel(**inputs):
    nc = get_nc()
    in_maps = [_host_prep(inputs, core) for core in range(B)]
    res = bass_utils.run_bass_kernel_spmd(nc, in_maps, core_ids=list(range(B)))
    outs = res.results
    enh_r = np.stack([outs[b]["enh_r"] for b in range(B)])
    enh_i = np.stack([outs[b]["enh_i"] for b in range(B)])
    u = np.stack([outs[b]["u_out"][0] for b in range(B)])
    return enh_r, enh_i, u


# revision 29
# speedup vs baseline: 1.0669x; 1.0475x over previous
"""DNN beamformer (MVDR with attention reference) on 8 Trainium2 NeuronCores.

Sharding: batch-parallel — core b handles batch b (B=8). Per core:
  - attention: psd_s -> feat -> MLP -> softmax -> u (1,6)
  - 257 independent 6x6 complex solves psd_n X = psd_s via Gauss-Jordan,
    vectorized with (b,f) pairs on SBUF partitions (3 groups of 120 f)
  - trace-normalize, steer by u -> beamforming weights
  - apply y = w^H x via block-diagonal PE matmuls (10 freqs per matmul)
"""
import sys
import numpy as np

for _p in ("/opt/trn_rl_repo",):
    if _p not in sys.path:
        sys.path.insert(0, _p)

import concourse.bacc as bacc
import concourse.mybir as mybir
import concourse.tile as tile
from concourse.bass_types import AP
from concourse import bass_utils

F32 = mybir.dt.float32
F32R = mybir.dt.float32r
Alu = mybir.AluOpType
Act = mybir.ActivationFunctionType
AX = mybir.AxisListType

B, F, C, T, ATT = 8, 257, 6, 1500, 320
FP = 120            # freqs per partition-group
G = 3               # partition groups (3*120 = 360 >= 257)
NU = 10             # freqs per PE block
NB = 26             # number of PE blocks (25*10 + 7 = 257)
NSC = 12            # 2 (re/im) * C rows per freq in the block-diag
TCH = [(0, 512), (512, 512), (1024, 476)]      # psum-bank-aligned T chunks
BCH = [(0, 5), (5, 10), (10, 14), (14, 18), (18, 22), (22, 26)]  # rhs block chunks
# 4 blocks per PSUM tile (PE out base partition must be 0/32/64/96)
SUP = [list(range(a, min(a + 4, NB))) for a in range(0, NB, 4)]

_CACHED = None


def _build():
    nc = bacc.Bacc("TRN2", target_bir_lowering=False, debug=False,
                   enable_asserts=False, num_devices=8)

    data_r = nc.dram_tensor("data_r", [F, C, T], F32R, kind="ExternalInput")
    data_i = nc.dram_tensor("data_i", [F, C, T], F32R, kind="ExternalInput")
    aug_r = nc.dram_tensor("aug_r", [128, G, 6, 12], F32, kind="ExternalInput")
    aug_i = nc.dram_tensor("aug_i", [128, G, 6, 12], F32, kind="ExternalInput")
    wmlp_in = nc.dram_tensor("wmlp_in", [128, G, ATT], F32, kind="ExternalInput")
    wg_rep = nc.dram_tensor("wg_rep", [6, ATT], F32, kind="ExternalInput")
    b_rep = nc.dram_tensor("b_rep", [6, ATT], F32, kind="ExternalInput")
    ones128 = nc.dram_tensor("ones128", [1, 128], F32, kind="ExternalInput")
    ident6 = nc.dram_tensor("ident6", [6, 6], F32, kind="ExternalInput")
    ident26 = nc.dram_tensor("ident26", [26, 26], F32, kind="ExternalInput")
    mask10_in = nc.dram_tensor("mask10_in", [120, NU], mybir.dt.uint8, kind="ExternalInput")

    enh_r = nc.dram_tensor("enh_r", [F, T], F32, kind="ExternalOutput")
    enh_i = nc.dram_tensor("enh_i", [F, T], F32, kind="ExternalOutput")
    u_out = nc.dram_tensor("u_out", [1, 6], F32, kind="ExternalOutput")

    with tile.TileContext(nc) as tc:
        _emit(tc, data_r, data_i, aug_r, aug_i, wmlp_in, wg_rep, b_rep,
              ones128, ident6, ident26, mask10_in, enh_r, enh_i, u_out)

    nc.compile()
    return nc


def _emit(tc, data_r, data_i, aug_r, aug_i, wmlp_in, wg_rep, b_rep,
          ones128, ident6, ident26, mask10_in, enh_r, enh_i, u_out):
    nc = tc.nc
    from contextlib import ExitStack
    ctx = ExitStack()

    cst = ctx.enter_context(tc.tile_pool(name="cst", bufs=1))
    small = ctx.enter_context(tc.tile_pool(name="small", bufs=1))
    tmp4 = ctx.enter_context(tc.tile_pool(name="tmp4", bufs=8))
    rhsp = ctx.enter_context(tc.tile_pool(name="rhsp", bufs=3))
    stgp = ctx.enter_context(tc.tile_pool(name="stgp", bufs=2))
    dramp = ctx.enter_context(tc.tile_pool(name="dramp", bufs=1, space="DRAM"))
    psum_s = ctx.enter_context(tc.tile_pool(name="psum_s", bufs=2, space="PSUM"))
    psum_m = ctx.enter_context(tc.tile_pool(name="psum_m", bufs=2, space="PSUM"))

    # ---------------- constant-ish loads ----------------
    augR = cst.tile([128, G, 6, 12], F32)
    augI = cst.tile([128, G, 6, 12], F32)
    wmlp = cst.tile([128, G, ATT], F32)
    wgr = cst.tile([6, ATT], F32)
    brp = cst.tile([6, ATT], F32)
    ones = cst.tile([1, 128], F32)
    id6 = cst.tile([6, 6], F32)
    id26 = cst.tile([26, 26], F32)
    mask10 = cst.tile([120, NU], mybir.dt.uint8)
    nc.scalar.dma_start(augR[:], aug_r[:])
    nc.scalar.dma_start(augI[:], aug_i[:])
    nc.sync.dma_start(wmlp[:], wmlp_in[:])
    nc.scalar.dma_start(wgr[:], wg_rep[:])
    nc.scalar.dma_start(brp[:], b_rep[:])
    nc.scalar.dma_start(ones[:], ones128[:])
    nc.scalar.dma_start(id6[:], ident6[:])
    nc.scalar.dma_start(id26[:], ident26[:])
    nc.scalar.dma_start(mask10[:], mask10_in[:])

    def diag_ap(t, col0):
        # [128, G, 6] strided diagonal of the 6x12 augmented row-major block
        base = t[:]
        return AP(tensor=base.tensor, offset=base.offset + col0,
                  ap=[list(base.ap[0]), list(base.ap[1]), [13, 6]])

    # ---------------- rhs data loads (start streaming immediately) ---------
    # rhs row k = 60s + 6u + c holds data[10b+u, c] (s=0 real, s=1 imag).
    # Within an s-half the partition index is affine in the source address
    # (addr = 1500*k), so one DMA covers 60 partitions -> all SDMA engines.
    rhs_tiles = {}
    for ci, (b0, b1) in enumerate(BCH):
        nbc = b1 - b0
        pitch = nbc * T
        rt = rhsp.tile([120, nbc * T], F32R, tag="rhs")
        rhs_tiles[ci] = rt
        rta = rt[:]
        nfull = nbc - 1 if b1 == NB else nbc  # block 25 has only 7 valid freqs
        for s, dram in ((0, data_r), (1, data_i)):
            eng = (nc.sync, nc.gpsimd, nc.scalar)[(2 * ci + s) % 3]
            dst = AP(tensor=rta.tensor, offset=rta.offset + (60 * s) * pitch,
                     ap=[[pitch, 60], [T, nfull], [1, T]])
            src = AP(tensor=dram[:].tensor, offset=(10 * b0) * C * T,
                     ap=[[T, 60], [10 * C * T, nfull], [1, T]])
            eng.dma_start(dst, src)
            if nfull < nbc:
                # block 25: 7 valid freqs (42 rows) + 18 dup rows from valid
                # data; the dup rows are annihilated by the zero lhsT columns.
                dst = AP(tensor=rta.tensor,
                         offset=rta.offset + (60 * s) * pitch + nfull * T,
                         ap=[[pitch, 42], [1, T]])
                src = AP(tensor=dram[:].tensor, offset=250 * C * T,
                         ap=[[T, 42], [1, T]])
                nc.sync.dma_start(dst, src)
                dst = AP(tensor=rta.tensor,
                         offset=rta.offset + (60 * s + 42) * pitch + nfull * T,
                         ap=[[pitch, 18], [1, T]])
                src = AP(tensor=dram[:].tensor, offset=247 * C * T,
                         ap=[[T, 18], [1, T]])
                nc.sync.dma_start(dst, src)

    # ---------------- attention: feat -> MLP -> softmax -> u ----------------
    rsR = small.tile([128, G, 6], F32)
    rsI = small.tile([128, G, 6], F32)
    sq = small.tile([128, G, 6], F32)
    sq2 = small.tile([128, G, 6], F32)
    feat = small.tile([128, G, 6], F32)
    nc.vector.tensor_reduce(rsR[:], augR[:, :, :, 6:12], AX.X, Alu.add)
    nc.vector.tensor_tensor(rsR[:], rsR[:], diag_ap(augR, 6), Alu.subtract)
    nc.vector.tensor_reduce(rsI[:], augI[:, :, :, 6:12], AX.X, Alu.add)
    nc.vector.tensor_tensor(rsI[:], rsI[:], diag_ap(augI, 6), Alu.subtract)
    nc.vector.tensor_tensor(sq[:], rsR[:], rsR[:], Alu.mult)
    nc.vector.tensor_tensor(sq2[:], rsI[:], rsI[:], Alu.mult)
    nc.vector.tensor_tensor(sq[:], sq[:], sq2[:], Alu.add)
    nc.scalar.activation(feat[:], sq[:], Act.Sqrt, bias=0.0, scale=1.0 / 25.0)

    mlp_ps = psum_s.tile([6, ATT], F32, tag="sp")
    for g in range(G):
        nc.tensor.matmul(mlp_ps[:], feat[:, g, :], wmlp[:, g, :],
                         start=(g == 0), stop=(g == G - 1))
    tb = small.tile([6, ATT], F32)
    nc.vector.tensor_tensor(tb[:], mlp_ps[:], brp[:], Alu.add)
    th = small.tile([6, ATT], F32)
    nc.scalar.activation(th[:], tb[:], Act.Tanh)
    tm = small.tile([6, ATT], F32)
    nc.vector.tensor_tensor(tm[:], th[:], wgr[:], Alu.mult)
    e_sb = small.tile([6, 1], F32)
    nc.vector.tensor_reduce(e_sb[:], tm[:], AX.X, Alu.add)

    eT_ps = psum_s.tile([1, 6], F32, tag="sp")
    nc.tensor.matmul(eT_ps[:], e_sb[:], id6[:], is_transpose=True,
                     start=True, stop=True)
    mx = small.tile([1, 1], F32)
    mb = small.tile([1, 1], F32)
    ex = small.tile([1, 6], F32)
    sm = small.tile([1, 1], F32)
    rc = small.tile([1, 1], F32)
    u_sb = small.tile([1, 6], F32)
    nc.vector.tensor_reduce(mx[:], eT_ps[:], AX.X, Alu.max)
    nc.vector.tensor_scalar_mul(mb[:], mx[:], -2.0)
    nc.scalar.activation(ex[:], eT_ps[:], Act.Exp, bias=mb[:], scale=2.0)
    nc.vector.tensor_reduce(sm[:], ex[:], AX.X, Alu.add)
    nc.vector.reciprocal(rc[:], sm[:])
    nc.vector.tensor_scalar_mul(u_sb[:], ex[:], rc[:])
    nc.scalar.dma_start(u_out[:], u_sb[:])

    urep_ps = psum_s.tile([128, 6], F32, tag="sp")
    nc.tensor.matmul(urep_ps[:], ones[:], u_sb[:], start=True, stop=True)
    u_rep = small.tile([128, 6], F32)
    nc.vector.tensor_copy(u_rep[:], urep_ps[:])

    # ---------------- Gauss-Jordan elimination (augmented [A | B]) ---------
    invd = small.tile([128, G, 6], F32)
    fr = small.tile([128, G, 6], F32)
    fi = small.tile([128, G, 6], F32)
    for i in range(6):
        w = 11 - i
        nc.vector.reciprocal(invd[:, :, i], augR[:, :, i, i])
        ird_b = invd[:, :, i].unsqueeze(2).broadcast_to([128, G, 6])
        nc.vector.tensor_tensor(fr[:], augR[:, :, :, i], ird_b, Alu.mult)
        nc.vector.tensor_tensor(fi[:], augI[:, :, :, i], ird_b, Alu.mult)
        nc.vector.memset(fr[:, :, i], 0.0)
        nc.vector.memset(fi[:, :, i], 0.0)
        rowR = augR[:, :, i, i + 1:].unsqueeze(2).broadcast_to([128, G, 6, w])
        rowI = augI[:, :, i, i + 1:].unsqueeze(2).broadcast_to([128, G, 6, w])
        fr_b = fr[:].unsqueeze(3).broadcast_to([128, G, 6, w])
        fi_b = fi[:].unsqueeze(3).broadcast_to([128, G, 6, w])
        t1 = tmp4.tile([128, G, 6, w], F32, tag="t1")
        t2 = tmp4.tile([128, G, 6, w], F32, tag="t2")
        t3 = tmp4.tile([128, G, 6, w], F32, tag="t3")
        t4 = tmp4.tile([128, G, 6, w], F32, tag="t4")
        nc.vector.tensor_tensor(t1[:], fr_b, rowR, Alu.mult)
        nc.vector.tensor_tensor(t2[:], fi_b, rowI, Alu.mult)
        nc.vector.tensor_tensor(t3[:], fr_b, rowI, Alu.mult)
        nc.vector.tensor_tensor(t4[:], fi_b, rowR, Alu.mult)
        nc.vector.tensor_tensor(augR[:, :, :, i + 1:], augR[:, :, :, i + 1:], t1[:], Alu.subtract)
        nc.vector.tensor_tensor(augR[:, :, :, i + 1:], augR[:, :, :, i + 1:], t2[:], Alu.add)
        nc.vector.tensor_tensor(augI[:, :, :, i + 1:], augI[:, :, :, i + 1:], t3[:], Alu.subtract)
        nc.vector.tensor_tensor(augI[:, :, :, i + 1:], augI[:, :, :, i + 1:], t4[:], Alu.subtract)

    # ---------------- trace, steering, weights ----------------
    t6a = small.tile([128, G, 6], F32)
    t6b = small.tile([128, G, 6], F32)
    trR = small.tile([128, G], F32)
    trI = small.tile([128, G], F32)
    nc.vector.tensor_tensor(t6a[:], diag_ap(augR, 6), invd[:], Alu.mult)
    nc.vector.tensor_reduce(trR[:], t6a[:], AX.X, Alu.add)
    nc.vector.tensor_tensor(t6b[:], diag_ap(augI, 6), invd[:], Alu.mult)
    nc.vector.tensor_reduce(trI[:], t6b[:], AX.X, Alu.add)

    u_b = u_rep[:].unsqueeze(1).unsqueeze(2).broadcast_to([128, G, 6, 6])
    t66 = tmp4.tile([128, G, 6, 6], F32, tag="t1")
    rawR = small.tile([128, G, 6], F32)
    rawI = small.tile([128, G, 6], F32)
    nc.vector.tensor_tensor(t66[:], augR[:, :, :, 6:12], u_b, Alu.mult)
    nc.vector.tensor_reduce(rawR[:], t66[:], AX.X, Alu.add)
    t66b = tmp4.tile([128, G, 6, 6], F32, tag="t2")
    nc.vector.tensor_tensor(t66b[:], augI[:, :, :, 6:12], u_b, Alu.mult)
    nc.vector.tensor_reduce(rawI[:], t66b[:], AX.X, Alu.add)
    nc.vector.tensor_tensor(rawR[:], rawR[:], invd[:], Alu.mult)
    nc.vector.tensor_tensor(rawI[:], rawI[:], invd[:], Alu.mult)

    trr = small.tile([128, G], F32)
    den = small.tile([128, G], F32)
    dn2 = small.tile([128, G], F32)
    rec = small.tile([128, G], F32)
    itr = small.tile([128, G], F32)
    iti = small.tile([128, G], F32)
    nc.vector.tensor_scalar_add(trr[:], trR[:], 1e-6)
    nc.vector.tensor_tensor(den[:], trr[:], trr[:], Alu.mult)
    nc.vector.tensor_tensor(dn2[:], trI[:], trI[:], Alu.mult)
    nc.vector.tensor_tensor(den[:], den[:], dn2[:], Alu.add)
    nc.vector.reciprocal(rec[:], den[:])
    nc.vector.tensor_tensor(itr[:], trr[:], rec[:], Alu.mult)
    nc.vector.scalar_tensor_tensor(iti[:], trI[:], -1.0, rec[:], Alu.mult, Alu.mult)

    itr_b = itr[:].unsqueeze(2).broadcast_to([128, G, 6])
    iti_b = iti[:].unsqueeze(2).broadcast_to([128, G, 6])
    # wcat[p, g, rho, sc]: rho=0 -> [a | b], rho=1 -> [-b | a]
    wcat = small.tile([128, G, 2, NSC], F32)
    m3 = small.tile([128, G, 6], F32)
    m4 = small.tile([128, G, 6], F32)
    nc.vector.tensor_tensor(m3[:], rawR[:], itr_b, Alu.mult)
    nc.vector.tensor_tensor(m4[:], rawI[:], iti_b, Alu.mult)
    nc.vector.tensor_tensor(wcat[:, :, 0, 0:6], m3[:], m4[:], Alu.subtract)   # a
    nc.vector.tensor_tensor(m3[:], rawR[:], iti_b, Alu.mult)
    nc.vector.tensor_tensor(m4[:], rawI[:], itr_b, Alu.mult)
    nc.vector.tensor_tensor(wcat[:, :, 0, 6:12], m3[:], m4[:], Alu.add)       # b
    nc.vector.tensor_scalar_mul(wcat[:, :, 1, 0:6], wcat[:, :, 0, 6:12], -1.0)  # -b
    nc.vector.tensor_copy(wcat[:, :, 1, 6:12], wcat[:, :, 0, 0:6])            # a

    # ------- block-diag lhsT: DRAM round-trip + PE transpose + masked copy --
    # bd3[f, rho, sc] = wcat[f%120, f//120, rho, sc]
    bd3 = dramp.tile([G * FP, 2, NSC], F32)
    bda = bd3[:]
    bdst = AP(tensor=bda.tensor, offset=bda.offset,
              ap=[[24, 120], [FP * 24, G], [1, 24]])
    nc.scalar.dma_start(bdst, wcat[0:120, :, :, :])

    # inb_rho[b, 12u+sc] = w[10b+u, rho, sc];  wT_rho[k, b] = inb_rho[b, k]
    lhsTf = cst.tile([120, NB * 32], F32)
    lhsT = cst.tile([120, NB * 32], F32R)
    nc.vector.memset(lhsTf[:], 0.0)
    lta = lhsTf[:]
    mask_b = mask10[:].unsqueeze(1).broadcast_to([120, NB, NU])
    # one contiguous load of bd3 (960B descriptors), then a strided DVE
    # shuffle into the transpose-input order k = 60s + 6u + c
    binb = small.tile([NB, NU * 24], F32)
    bsrc = AP(tensor=bda.tensor, offset=bda.offset,
              ap=[[NU * 24, NB], [1, NU * 24]])
    nc.scalar.dma_start(binb[:], bsrc)
    for rho in range(2):
        inb = small.tile([NB, FP], F32, tag=f"inb{rho}")
        iap = AP(tensor=inb[:].tensor, offset=inb[:].offset,
                 ap=[list(inb[:].ap[0]), [60, 2], [6, NU], [1, 6]])
        bap = AP(tensor=binb[:].tensor, offset=binb[:].offset + NSC * rho,
                 ap=[list(binb[:].ap[0]), [6, 2], [24, NU], [1, 6]])
        nc.vector.tensor_copy(iap, bap)
        psT = psum_s.tile([120, NB], F32, tag="sp")
        nc.tensor.matmul(psT[:], inb[:], id26[:], is_transpose=True,
                         start=True, stop=True)
        wT = small.tile([120, NB], F32, tag=f"wT{rho}")
        nc.vector.tensor_copy(wT[:], psT[:])
        out_ap = AP(tensor=lta.tensor, offset=lta.offset + 16 * rho,
                    ap=[[NB * 32, 120], [32, NB], [1, NU]])
        data_ap = wT[:].unsqueeze(2).broadcast_to([120, NB, NU])
        nc.vector.copy_predicated(out_ap, mask_b, data_ap)
    nc.vector.tensor_copy(lhsT[:], lhsTf[:])

    # ---------------- beamforming application (PE block-diag matmuls) ------
    # fp32r matmuls require dst start_partition == 0 -> one PSUM tile per
    # block; outputs of 6 blocks are staged side by side so each out-DMA
    # spans many descriptors (keeps the SDMA engines fed).
    for s0 in range(0, NB, 6):
        sblocks = list(range(s0, min(s0 + 6, NB)))
        stg = stgp.tile([32, 6 * 1500], F32, tag="stg")
        for b in sblocks:
            j = b - s0
            ci = next(k for k, (a0, a1) in enumerate(BCH) if a0 <= b < a1)
            bl = b - BCH[ci][0]
            rt = rhs_tiles[ci]
            ps = psum_m.tile([32, 1536], F32, tag="mac")
            for (t0, tn) in TCH:
                lhs_s = lhsT[:, 32 * b:32 * (b + 1)]
                rhs_s = rt[:, bl * T + t0: bl * T + t0 + tn]
                nc.tensor.matmul(ps[:, t0:t0 + tn], lhs_s, rhs_s,
                                 start=True, stop=True)
            # split the PSUM->SBUF copy across ACT and DVE so neither
            # engine paces the PE
            nc.scalar.activation(stg[0:26, j * 1500: j * 1500 + 1024],
                                 ps[0:26, 0:1024], Act.Copy)
            nc.vector.tensor_copy(stg[0:26, j * 1500 + 1024: j * 1500 + T],
                                  ps[0:26, 1024:T])
        nfb = len([b for b in sblocks if b != NB - 1])  # full blocks
        sta = stg[:]
        for rho, dram in ((0, enh_r), (1, enh_i)):
            eng = (nc.sync, nc.gpsimd, nc.scalar)[(2 * (s0 // 6) + rho) % 3]
            src = AP(tensor=sta.tensor, offset=sta.offset + (16 * rho) * (6 * 1500),
                     ap=[[6 * 1500, NU], [1500, nfb], [1, T]])
            dst = AP(tensor=dram[:].tensor, offset=(10 * s0) * T,
                     ap=[[T, NU], [10 * T, nfb], [1, T]])
            eng.dma_start(dst, src)
        if NB - 1 in sblocks:
            j = NB - 1 - s0
            for rho, dram in ((0, enh_r), (1, enh_i)):
                src = AP(tensor=sta.tensor,
                         offset=sta.offset + (16 * rho) * (6 * 1500) + j * 1500,
                         ap=[[6 * 1500, 7], [1, T]])
                dst = AP(tensor=dram[:].tensor, offset=(10 * (NB - 1)) * T,
                         ap=[[T, 7], [1, T]])
                nc.sync.dma_start(dst, src)

    ctx.close()


def _host_prep(inputs, core):
    """Build the per-core input map from the full problem inputs."""
    psd_s_r = np.asarray(inputs["psd_s_r"][core])
    psd_s_i = np.asarray(inputs["psd_s_i"][core])
    psd_n_r = np.asarray(inputs["psd_n_r"][core])
    psd_n_i = np.asarray(inputs["psd_n_i"][core])
    w_mlp = np.asarray(inputs["w_mlp"])
    b_mlp = np.asarray(inputs["b_mlp"])
    w_gvec = np.asarray(inputs["w_gvec"])

    aug_r = np.zeros((G, 128, 6, 12), np.float32)
    aug_i = np.zeros((G, 128, 6, 12), np.float32)
    aug_r[:, :, :, 0:6] = np.eye(6, dtype=np.float32)  # pad A with identity
    fpad_r = np.zeros((G * FP, 6, 6), np.float32)
    fpad_r[:F] = psd_n_r
    fpad_i = np.zeros((G * FP, 6, 6), np.float32)
    fpad_i[:F] = psd_n_i
    fpad_r[F:] = np.eye(6, dtype=np.float32)
    aug_r[:, :FP, :, 0:6] = fpad_r.reshape(G, FP, 6, 6)
    aug_i[:, :FP, :, 0:6] = fpad_i.reshape(G, FP, 6, 6)
    spad_r = np.zeros((G * FP, 6, 6), np.float32)
    spad_r[:F] = psd_s_r
    spad_i = np.zeros((G * FP, 6, 6), np.float32)
    spad_i[:F] = psd_s_i
    aug_r[:, :FP, :, 6:12] = spad_r.reshape(G, FP, 6, 6)
    aug_i[:, :FP, :, 6:12] = spad_i.reshape(G, FP, 6, 6)
    aug_r = np.ascontiguousarray(aug_r.transpose(1, 0, 2, 3))
    aug_i = np.ascontiguousarray(aug_i.transpose(1, 0, 2, 3))

    wmlp = np.zeros((G, 128, ATT), np.float32)
    wpad = np.zeros((G * FP, ATT), np.float32)
    wpad[:F] = w_mlp
    wmlp[:, :FP] = wpad.reshape(G, FP, ATT)
    wmlp = np.ascontiguousarray(wmlp.transpose(1, 0, 2))

    return {
        "data_r": np.ascontiguousarray(inputs["data_r"][core]),
        "data_i": np.ascontiguousarray(inputs["data_i"][core]),
        "aug_r": aug_r,
        "aug_i": aug_i,
        "wmlp_in": wmlp,
        "wg_rep": np.ascontiguousarray(
            np.broadcast_to(w_gvec[:, 0][None, :], (6, ATT))).astype(np.float32),
        "b_rep": np.ascontiguousarray(
            np.broadcast_to(b_mlp[None, :], (6, ATT))).astype(np.float32),
        "ones128": np.ones((1, 128), np.float32),
        "ident6": np.eye(6, dtype=np.float32),
        "ident26": np.eye(26, dtype=np.float32),
        "mask10_in": (np.arange(NU)[None, :] ==
                      ((np.arange(120) % 60) // 6)[:, None]).astype(np.uint8),
    }


def get_nc():
    global _CACHED
    if _CACHED is None:
        _CACHED = _build()
    return _CACHED


def kernel(**inputs):
    nc = get_nc()
    in_maps = [_host_prep(inputs, core) for core in range(B)]
    res = bass_utils.run_bass_kernel_spmd(nc, in_maps, core_ids=list(range(B)))
    outs = res.results
    enh_r = np.stack([outs[b]["enh_r"] for b in range(B)])
    enh_i = np.stack([outs[b]["enh_i"] for b in range(B)])
    u = np.stack([outs[b]["u_out"][0] for b in range(B)])
    return enh_r, enh_i, u


# revision 30
# speedup vs baseline: 1.1134x; 1.0435x over previous
"""DNN beamformer (MVDR with attention reference) on 8 Trainium2 NeuronCores.

Sharding: batch-parallel — core b handles batch b (B=8). Per core:
  - attention: psd_s -> feat -> MLP -> softmax -> u (1,6)
  - 257 independent 6x6 complex solves psd_n X = psd_s via Gauss-Jordan,
    vectorized with (b,f) pairs on SBUF partitions (3 groups of 120 f)
  - trace-normalize, steer by u -> beamforming weights
  - apply y = w^H x via block-diagonal PE matmuls (10 freqs per matmul)
"""
import sys
import numpy as np

for _p in ("/opt/trn_rl_repo",):
    if _p not in sys.path:
        sys.path.insert(0, _p)

import concourse.bacc as bacc
import concourse.mybir as mybir
import concourse.tile as tile
from concourse.bass_types import AP
from concourse import bass_utils

F32 = mybir.dt.float32
F32R = mybir.dt.float32r
Alu = mybir.AluOpType
Act = mybir.ActivationFunctionType
AX = mybir.AxisListType

B, F, C, T, ATT = 8, 257, 6, 1500, 320
FP = 120            # freqs per partition-group
G = 3               # partition groups (3*120 = 360 >= 257)
NU = 10             # freqs per PE block
NB = 26             # number of PE blocks (25*10 + 7 = 257)
NSC = 12            # 2 (re/im) * C rows per freq in the block-diag
TCH = [(0, 512), (512, 512), (1024, 476)]      # psum-bank-aligned T chunks
BCH = [(0, 4), (4, 8), (8, 12), (12, 16), (16, 20), (20, 23), (23, 26)]  # rhs block chunks
# 4 blocks per PSUM tile (PE out base partition must be 0/32/64/96)
SUP = [list(range(a, min(a + 4, NB))) for a in range(0, NB, 4)]

_CACHED = None


def _build():
    nc = bacc.Bacc("TRN2", target_bir_lowering=False, debug=False,
                   enable_asserts=False, num_devices=8)

    data_r = nc.dram_tensor("data_r", [F, C, T], F32R, kind="ExternalInput")
    data_i = nc.dram_tensor("data_i", [F, C, T], F32R, kind="ExternalInput")
    aug_r = nc.dram_tensor("aug_r", [128, G, 6, 12], F32, kind="ExternalInput")
    aug_i = nc.dram_tensor("aug_i", [128, G, 6, 12], F32, kind="ExternalInput")
    wmlp_in = nc.dram_tensor("wmlp_in", [128, G, ATT], F32, kind="ExternalInput")
    wg_rep = nc.dram_tensor("wg_rep", [6, ATT], F32, kind="ExternalInput")
    b_rep = nc.dram_tensor("b_rep", [6, ATT], F32, kind="ExternalInput")
    ones128 = nc.dram_tensor("ones128", [1, 128], F32, kind="ExternalInput")
    ident6 = nc.dram_tensor("ident6", [6, 6], F32, kind="ExternalInput")
    ident26 = nc.dram_tensor("ident26", [26, 26], F32, kind="ExternalInput")
    mask10_in = nc.dram_tensor("mask10_in", [120, NU], mybir.dt.uint8, kind="ExternalInput")

    enh_r = nc.dram_tensor("enh_r", [F, T], F32, kind="ExternalOutput")
    enh_i = nc.dram_tensor("enh_i", [F, T], F32, kind="ExternalOutput")
    u_out = nc.dram_tensor("u_out", [1, 6], F32, kind="ExternalOutput")

    with tile.TileContext(nc) as tc:
        _emit(tc, data_r, data_i, aug_r, aug_i, wmlp_in, wg_rep, b_rep,
              ones128, ident6, ident26, mask10_in, enh_r, enh_i, u_out)

    nc.compile()
    return nc


def _emit(tc, data_r, data_i, aug_r, aug_i, wmlp_in, wg_rep, b_rep,
          ones128, ident6, ident26, mask10_in, enh_r, enh_i, u_out):
    nc = tc.nc
    from contextlib import ExitStack
    ctx = ExitStack()

    cst = ctx.enter_context(tc.tile_pool(name="cst", bufs=1))
    small = ctx.enter_context(tc.tile_pool(name="small", bufs=1))
    tmp4 = ctx.enter_context(tc.tile_pool(name="tmp4", bufs=8))
    rhsp = ctx.enter_context(tc.tile_pool(name="rhsp", bufs=4))
    stgp = ctx.enter_context(tc.tile_pool(name="stgp", bufs=2))
    dramp = ctx.enter_context(tc.tile_pool(name="dramp", bufs=1, space="DRAM"))
    psum_s = ctx.enter_context(tc.tile_pool(name="psum_s", bufs=2, space="PSUM"))
    psum_m = ctx.enter_context(tc.tile_pool(name="psum_m", bufs=2, space="PSUM"))

    # ---------------- constant-ish loads ----------------
    augR = cst.tile([128, G, 6, 12], F32)
    augI = cst.tile([128, G, 6, 12], F32)
    wmlp = cst.tile([128, G, ATT], F32)
    wgr = cst.tile([6, ATT], F32)
    brp = cst.tile([6, ATT], F32)
    ones = cst.tile([1, 128], F32)
    id6 = cst.tile([6, 6], F32)
    id26 = cst.tile([26, 26], F32)
    mask10 = cst.tile([120, NU], mybir.dt.uint8)
    nc.scalar.dma_start(augR[:], aug_r[:])
    nc.scalar.dma_start(augI[:], aug_i[:])
    nc.sync.dma_start(wmlp[:], wmlp_in[:])
    nc.scalar.dma_start(wgr[:], wg_rep[:])
    nc.scalar.dma_start(brp[:], b_rep[:])
    nc.scalar.dma_start(ones[:], ones128[:])
    nc.scalar.dma_start(id6[:], ident6[:])
    nc.scalar.dma_start(id26[:], ident26[:])
    nc.scalar.dma_start(mask10[:], mask10_in[:])

    def diag_ap(t, col0):
        # [128, G, 6] strided diagonal of the 6x12 augmented row-major block
        base = t[:]
        return AP(tensor=base.tensor, offset=base.offset + col0,
                  ap=[list(base.ap[0]), list(base.ap[1]), [13, 6]])

    # ---------------- attention: feat -> MLP -> softmax -> u ----------------
    rsR = small.tile([128, G, 6], F32)
    rsI = small.tile([128, G, 6], F32)
    sq = small.tile([128, G, 6], F32)
    sq2 = small.tile([128, G, 6], F32)
    feat = small.tile([128, G, 6], F32)
    nc.vector.tensor_reduce(rsR[:], augR[:, :, :, 6:12], AX.X, Alu.add)
    nc.vector.tensor_tensor(rsR[:], rsR[:], diag_ap(augR, 6), Alu.subtract)
    nc.vector.tensor_reduce(rsI[:], augI[:, :, :, 6:12], AX.X, Alu.add)
    nc.vector.tensor_tensor(rsI[:], rsI[:], diag_ap(augI, 6), Alu.subtract)
    nc.vector.tensor_tensor(sq[:], rsR[:], rsR[:], Alu.mult)
    nc.vector.tensor_tensor(sq2[:], rsI[:], rsI[:], Alu.mult)
    nc.vector.tensor_tensor(sq[:], sq[:], sq2[:], Alu.add)
    nc.scalar.activation(feat[:], sq[:], Act.Sqrt, bias=0.0, scale=1.0 / 25.0)


    # ---------------- rhs data loads (start streaming immediately) ---------
    # rhs row k = 60s + 6u + c holds data[10b+u, c] (s=0 real, s=1 imag).
    # Within an s-half the partition index is affine in the source address
    # (addr = 1500*k), so one DMA covers 60 partitions -> all SDMA engines.
    rhs_tiles = {}
    for ci, (b0, b1) in enumerate(BCH):
        nbc = b1 - b0
        pitch = nbc * T
        rt = rhsp.tile([120, nbc * T], F32R, tag="rhs")
        rhs_tiles[ci] = rt
        rta = rt[:]
        nfull = nbc - 1 if b1 == NB else nbc  # block 25 has only 7 valid freqs
        for s, dram in ((0, data_r), (1, data_i)):
            eng = (nc.sync, nc.gpsimd, nc.scalar)[(2 * ci + s) % 3]
            dst = AP(tensor=rta.tensor, offset=rta.offset + (60 * s) * pitch,
                     ap=[[pitch, 60], [T, nfull], [1, T]])
            src = AP(tensor=dram[:].tensor, offset=(10 * b0) * C * T,
                     ap=[[T, 60], [10 * C * T, nfull], [1, T]])
            eng.dma_start(dst, src)
            if nfull < nbc:
                # block 25: 7 valid freqs (42 rows) + 18 dup rows from valid
                # data; the dup rows are annihilated by the zero lhsT columns.
                dst = AP(tensor=rta.tensor,
                         offset=rta.offset + (60 * s) * pitch + nfull * T,
                         ap=[[pitch, 42], [1, T]])
                src = AP(tensor=dram[:].tensor, offset=250 * C * T,
                         ap=[[T, 42], [1, T]])
                nc.sync.dma_start(dst, src)
                dst = AP(tensor=rta.tensor,
                         offset=rta.offset + (60 * s + 42) * pitch + nfull * T,
                         ap=[[pitch, 18], [1, T]])
                src = AP(tensor=dram[:].tensor, offset=247 * C * T,
                         ap=[[T, 18], [1, T]])
                nc.sync.dma_start(dst, src)

    mlp_ps = psum_s.tile([6, ATT], F32, tag="sp")
    for g in range(G):
        nc.tensor.matmul(mlp_ps[:], feat[:, g, :], wmlp[:, g, :],
                         start=(g == 0), stop=(g == G - 1))
    tb = small.tile([6, ATT], F32)
    nc.vector.tensor_tensor(tb[:], mlp_ps[:], brp[:], Alu.add)
    th = small.tile([6, ATT], F32)
    nc.scalar.activation(th[:], tb[:], Act.Tanh)
    tm = small.tile([6, ATT], F32)
    nc.vector.tensor_tensor(tm[:], th[:], wgr[:], Alu.mult)
    e_sb = small.tile([6, 1], F32)
    nc.vector.tensor_reduce(e_sb[:], tm[:], AX.X, Alu.add)

    eT_ps = psum_s.tile([1, 6], F32, tag="sp")
    nc.tensor.matmul(eT_ps[:], e_sb[:], id6[:], is_transpose=True,
                     start=True, stop=True)
    mx = small.tile([1, 1], F32)
    mb = small.tile([1, 1], F32)
    ex = small.tile([1, 6], F32)
    sm = small.tile([1, 1], F32)
    rc = small.tile([1, 1], F32)
    u_sb = small.tile([1, 6], F32)
    nc.vector.tensor_reduce(mx[:], eT_ps[:], AX.X, Alu.max)
    nc.vector.tensor_scalar_mul(mb[:], mx[:], -2.0)
    nc.scalar.activation(ex[:], eT_ps[:], Act.Exp, bias=mb[:], scale=2.0)
    nc.vector.tensor_reduce(sm[:], ex[:], AX.X, Alu.add)
    nc.vector.reciprocal(rc[:], sm[:])
    nc.vector.tensor_scalar_mul(u_sb[:], ex[:], rc[:])
    nc.scalar.dma_start(u_out[:], u_sb[:])

    urep_ps = psum_s.tile([128, 6], F32, tag="sp")
    nc.tensor.matmul(urep_ps[:], ones[:], u_sb[:], start=True, stop=True)
    u_rep = small.tile([128, 6], F32)
    nc.vector.tensor_copy(u_rep[:], urep_ps[:])

    # ---------------- Gauss-Jordan elimination (augmented [A | B]) ---------
    invd = small.tile([128, G, 6], F32)
    fr = small.tile([128, G, 6], F32)
    fi = small.tile([128, G, 6], F32)
    for i in range(6):
        w = 11 - i
        nc.vector.reciprocal(invd[:, :, i], augR[:, :, i, i])
        ird_b = invd[:, :, i].unsqueeze(2).broadcast_to([128, G, 6])
        nc.vector.tensor_tensor(fr[:], augR[:, :, :, i], ird_b, Alu.mult)
        nc.vector.tensor_tensor(fi[:], augI[:, :, :, i], ird_b, Alu.mult)
        nc.vector.memset(fr[:, :, i], 0.0)
        nc.vector.memset(fi[:, :, i], 0.0)
        rowR = augR[:, :, i, i + 1:].unsqueeze(2).broadcast_to([128, G, 6, w])
        rowI = augI[:, :, i, i + 1:].unsqueeze(2).broadcast_to([128, G, 6, w])
        fr_b = fr[:].unsqueeze(3).broadcast_to([128, G, 6, w])
        fi_b = fi[:].unsqueeze(3).broadcast_to([128, G, 6, w])
        t1 = tmp4.tile([128, G, 6, w], F32, tag="t1")
        t2 = tmp4.tile([128, G, 6, w], F32, tag="t2")
        t3 = tmp4.tile([128, G, 6, w], F32, tag="t3")
        t4 = tmp4.tile([128, G, 6, w], F32, tag="t4")
        nc.vector.tensor_tensor(t1[:], fr_b, rowR, Alu.mult)
        nc.vector.tensor_tensor(t2[:], fi_b, rowI, Alu.mult)
        nc.vector.tensor_tensor(t3[:], fr_b, rowI, Alu.mult)
        nc.vector.tensor_tensor(t4[:], fi_b, rowR, Alu.mult)
        nc.vector.tensor_tensor(augR[:, :, :, i + 1:], augR[:, :, :, i + 1:], t1[:], Alu.subtract)
        nc.vector.tensor_tensor(augR[:, :, :, i + 1:], augR[:, :, :, i + 1:], t2[:], Alu.add)
        nc.vector.tensor_tensor(augI[:, :, :, i + 1:], augI[:, :, :, i + 1:], t3[:], Alu.subtract)
        nc.vector.tensor_tensor(augI[:, :, :, i + 1:], augI[:, :, :, i + 1:], t4[:], Alu.subtract)

    # ---------------- trace, steering, weights ----------------
    t6a = small.tile([128, G, 6], F32)
    t6b = small.tile([128, G, 6], F32)
    trR = small.tile([128, G], F32)
    trI = small.tile([128, G], F32)
    nc.vector.tensor_tensor(t6a[:], diag_ap(augR, 6), invd[:], Alu.mult)
    nc.vector.tensor_reduce(trR[:], t6a[:], AX.X, Alu.add)
    nc.vector.tensor_tensor(t6b[:], diag_ap(augI, 6), invd[:], Alu.mult)
    nc.vector.tensor_reduce(trI[:], t6b[:], AX.X, Alu.add)

    u_b = u_rep[:].unsqueeze(1).unsqueeze(2).broadcast_to([128, G, 6, 6])
    t66 = tmp4.tile([128, G, 6, 6], F32, tag="t1")
    rawR = small.tile([128, G, 6], F32)
    rawI = small.tile([128, G, 6], F32)
    nc.vector.tensor_tensor(t66[:], augR[:, :, :, 6:12], u_b, Alu.mult)
    nc.vector.tensor_reduce(rawR[:], t66[:], AX.X, Alu.add)
    t66b = tmp4.tile([128, G, 6, 6], F32, tag="t2")
    nc.vector.tensor_tensor(t66b[:], augI[:, :, :, 6:12], u_b, Alu.mult)
    nc.vector.tensor_reduce(rawI[:], t66b[:], AX.X, Alu.add)
    nc.vector.tensor_tensor(rawR[:], rawR[:], invd[:], Alu.mult)
    nc.vector.tensor_tensor(rawI[:], rawI[:], invd[:], Alu.mult)

    trr = small.tile([128, G], F32)
    den = small.tile([128, G], F32)
    dn2 = small.tile([128, G], F32)
    rec = small.tile([128, G], F32)
    itr = small.tile([128, G], F32)
    iti = small.tile([128, G], F32)
    nc.vector.tensor_scalar_add(trr[:], trR[:], 1e-6)
    nc.vector.tensor_tensor(den[:], trr[:], trr[:], Alu.mult)
    nc.vector.tensor_tensor(dn2[:], trI[:], trI[:], Alu.mult)
    nc.vector.tensor_tensor(den[:], den[:], dn2[:], Alu.add)
    nc.vector.reciprocal(rec[:], den[:])
    nc.vector.tensor_tensor(itr[:], trr[:], rec[:], Alu.mult)
    nc.vector.scalar_tensor_tensor(iti[:], trI[:], -1.0, rec[:], Alu.mult, Alu.mult)

    itr_b = itr[:].unsqueeze(2).broadcast_to([128, G, 6])
    iti_b = iti[:].unsqueeze(2).broadcast_to([128, G, 6])
    # wcat[p, g, rho, sc]: rho=0 -> [a | b], rho=1 -> [-b | a]
    wcat = small.tile([128, G, 2, NSC], F32)
    m3 = small.tile([128, G, 6], F32)
    m4 = small.tile([128, G, 6], F32)
    nc.vector.tensor_tensor(m3[:], rawR[:], itr_b, Alu.mult)
    nc.vector.tensor_tensor(m4[:], rawI[:], iti_b, Alu.mult)
    nc.vector.tensor_tensor(wcat[:, :, 0, 0:6], m3[:], m4[:], Alu.subtract)   # a
    nc.vector.tensor_tensor(m3[:], rawR[:], iti_b, Alu.mult)
    nc.vector.tensor_tensor(m4[:], rawI[:], itr_b, Alu.mult)
    nc.vector.tensor_tensor(wcat[:, :, 0, 6:12], m3[:], m4[:], Alu.add)       # b
    nc.vector.tensor_scalar_mul(wcat[:, :, 1, 0:6], wcat[:, :, 0, 6:12], -1.0)  # -b
    nc.vector.tensor_copy(wcat[:, :, 1, 6:12], wcat[:, :, 0, 0:6])            # a

    # ------- block-diag lhsT: DRAM round-trip + PE transpose + masked copy --
    # bd3[f, rho, sc] = wcat[f%120, f//120, rho, sc]
    bd3 = dramp.tile([G * FP, 2, NSC], F32)
    bda = bd3[:]
    bdst = AP(tensor=bda.tensor, offset=bda.offset,
              ap=[[24, 120], [FP * 24, G], [1, 24]])
    nc.scalar.dma_start(bdst, wcat[0:120, :, :, :])

    # inb_rho[b, 12u+sc] = w[10b+u, rho, sc];  wT_rho[k, b] = inb_rho[b, k]
    lhsTf = cst.tile([120, NB * 32], F32)
    lhsT = cst.tile([120, NB * 32], F32R)
    nc.vector.memset(lhsTf[:], 0.0)
    lta = lhsTf[:]
    mask_b = mask10[:].unsqueeze(1).broadcast_to([120, NB, NU])
    # one contiguous load of bd3 (960B descriptors), then a strided DVE
    # shuffle into the transpose-input order k = 60s + 6u + c
    binb = small.tile([NB, NU * 24], F32)
    bsrc = AP(tensor=bda.tensor, offset=bda.offset,
              ap=[[NU * 24, NB], [1, NU * 24]])
    nc.scalar.dma_start(binb[:], bsrc)
    for rho in range(2):
        inb = small.tile([NB, FP], F32, tag=f"inb{rho}")
        iap = AP(tensor=inb[:].tensor, offset=inb[:].offset,
                 ap=[list(inb[:].ap[0]), [60, 2], [6, NU], [1, 6]])
        bap = AP(tensor=binb[:].tensor, offset=binb[:].offset + NSC * rho,
                 ap=[list(binb[:].ap[0]), [6, 2], [24, NU], [1, 6]])
        nc.vector.tensor_copy(iap, bap)
        psT = psum_s.tile([120, NB], F32, tag="sp")
        nc.tensor.matmul(psT[:], inb[:], id26[:], is_transpose=True,
                         start=True, stop=True)
        wT = small.tile([120, NB], F32, tag=f"wT{rho}")
        nc.vector.tensor_copy(wT[:], psT[:])
        out_ap = AP(tensor=lta.tensor, offset=lta.offset + 16 * rho,
                    ap=[[NB * 32, 120], [32, NB], [1, NU]])
        data_ap = wT[:].unsqueeze(2).broadcast_to([120, NB, NU])
        nc.vector.copy_predicated(out_ap, mask_b, data_ap)
    nc.vector.tensor_copy(lhsT[:], lhsTf[:])

    # ---------------- beamforming application (PE block-diag matmuls) ------
    # fp32r matmuls require dst start_partition == 0 -> one PSUM tile per
    # block; outputs of 6 blocks are staged side by side so each out-DMA
    # spans many descriptors (keeps the SDMA engines fed).
    for s0 in range(0, NB, 3):
        sblocks = list(range(s0, min(s0 + 3, NB)))
        stg = stgp.tile([32, 3 * 1500], F32, tag="stg")
        for b in sblocks:
            j = b - s0
            ci = next(k for k, (a0, a1) in enumerate(BCH) if a0 <= b < a1)
            bl = b - BCH[ci][0]
            rt = rhs_tiles[ci]
            ps = psum_m.tile([32, 1536], F32, tag="mac")
            for (t0, tn) in TCH:
                lhs_s = lhsT[:, 32 * b:32 * (b + 1)]
                rhs_s = rt[:, bl * T + t0: bl * T + t0 + tn]
                nc.tensor.matmul(ps[:, t0:t0 + tn], lhs_s, rhs_s,
                                 start=True, stop=True)
            # split the PSUM->SBUF copy across ACT and DVE so neither
            # engine paces the PE
            nc.scalar.activation(stg[0:26, j * 1500: j * 1500 + 1024],
                                 ps[0:26, 0:1024], Act.Copy)
            nc.vector.tensor_copy(stg[0:26, j * 1500 + 1024: j * 1500 + T],
                                  ps[0:26, 1024:T])
        nfb = len([b for b in sblocks if b != NB - 1])  # full blocks
        sta = stg[:]
        for rho, dram in ((0, enh_r), (1, enh_i)):
            eng = (nc.sync, nc.gpsimd, nc.scalar)[(2 * (s0 // 3) + rho) % 3]
            src = AP(tensor=sta.tensor, offset=sta.offset + (16 * rho) * (3 * 1500),
                     ap=[[3 * 1500, NU], [1500, nfb], [1, T]])
            dst = AP(tensor=dram[:].tensor, offset=(10 * s0) * T,
                     ap=[[T, NU], [10 * T, nfb], [1, T]])
            eng.dma_start(dst, src)
        if NB - 1 in sblocks:
            j = NB - 1 - s0
            for rho, dram in ((0, enh_r), (1, enh_i)):
                src = AP(tensor=sta.tensor,
                         offset=sta.offset + (16 * rho) * (3 * 1500) + j * 1500,
                         ap=[[3 * 1500, 7], [1, T]])
                dst = AP(tensor=dram[:].tensor, offset=(10 * (NB - 1)) * T,
                         ap=[[T, 7], [1, T]])
                nc.sync.dma_start(dst, src)

    ctx.close()


def _host_prep(inputs, core):
    """Build the per-core input map from the full problem inputs."""
    psd_s_r = np.asarray(inputs["psd_s_r"][core])
    psd_s_i = np.asarray(inputs["psd_s_i"][core])
    psd_n_r = np.asarray(inputs["psd_n_r"][core])
    psd_n_i = np.asarray(inputs["psd_n_i"][core])
    w_mlp = np.asarray(inputs["w_mlp"])
    b_mlp = np.asarray(inputs["b_mlp"])
    w_gvec = np.asarray(inputs["w_gvec"])

    aug_r = np.zeros((G, 128, 6, 12), np.float32)
    aug_i = np.zeros((G, 128, 6, 12), np.float32)
    aug_r[:, :, :, 0:6] = np.eye(6, dtype=np.float32)  # pad A with identity
    fpad_r = np.zeros((G * FP, 6, 6), np.float32)
    fpad_r[:F] = psd_n_r
    fpad_i = np.zeros((G * FP, 6, 6), np.float32)
    fpad_i[:F] = psd_n_i
    fpad_r[F:] = np.eye(6, dtype=np.float32)
    aug_r[:, :FP, :, 0:6] = fpad_r.reshape(G, FP, 6, 6)
    aug_i[:, :FP, :, 0:6] = fpad_i.reshape(G, FP, 6, 6)
    spad_r = np.zeros((G * FP, 6, 6), np.float32)
    spad_r[:F] = psd_s_r
    spad_i = np.zeros((G * FP, 6, 6), np.float32)
    spad_i[:F] = psd_s_i
    aug_r[:, :FP, :, 6:12] = spad_r.reshape(G, FP, 6, 6)
    aug_i[:, :FP, :, 6:12] = spad_i.reshape(G, FP, 6, 6)
    aug_r = np.ascontiguousarray(aug_r.transpose(1, 0, 2, 3))
    aug_i = np.ascontiguousarray(aug_i.transpose(1, 0, 2, 3))

    wmlp = np.zeros((G, 128, ATT), np.float32)
    wpad = np.zeros((G * FP, ATT), np.float32)
    wpad[:F] = w_mlp
    wmlp[:, :FP] = wpad.reshape(G, FP, ATT)
    wmlp = np.ascontiguousarray(wmlp.transpose(1, 0, 2))

    return {
        "data_r": np.ascontiguousarray(inputs["data_r"][core]),
        "data_i": np.ascontiguousarray(inputs["data_i"][core]),
        "aug_r": aug_r,
        "aug_i": aug_i,
        "wmlp_in": wmlp,
        "wg_rep": np.ascontiguousarray(
            np.broadcast_to(w_gvec[:, 0][None, :], (6, ATT))).astype(np.float32),
        "b_rep": np.ascontiguousarray(
            np.broadcast_to(b_mlp[None, :], (6, ATT))).astype(np.float32),
        "ones128": np.ones((1, 128), np.float32),
        "ident6": np.eye(6, dtype=np.float32),
        "ident26": np.eye(26, dtype=np.float32),
        "mask10_in": (np.arange(NU)[None, :] ==
                      ((np.arange(120) % 60) // 6)[:, None]).astype(np.uint8),
    }


def get_nc():
    global _CACHED
    if _CACHED is None:
        _CACHED = _build()
    return _CACHED


def kernel(**inputs):
    nc = get_nc()
    in_maps = [_host_prep(inputs, core) for core in range(B)]
    res = bass_utils.run_bass_kernel_spmd(nc, in_maps, core_ids=list(range(B)))
    outs = res.results
    enh_r = np.stack([outs[b]["enh_r"] for b in range(B)])
    enh_i = np.stack([outs[b]["enh_i"] for b in range(B)])
    u = np.stack([outs[b]["u_out"][0] for b in range(B)])
    return enh_r, enh_i, u
